# revision 5
# baseline (speedup 1.0000x reference)
"""Distilled-KL loss head on 8 TRN2 NeuronCores — v3.

Math (validated vs the jax reference, see numsim.py):
  For batch row r, with x = teacher logits (even r) / student (odd r), y the
  other tensor, the per-row term is
      rowval = -(1/mask_tot) * sum_t mask_t * sct_t * sum_v P~_v * f_v
  where P~ = e4m3(exp(x - 1))   [fp8 prob cache, written free by pass-1 exp]
        sct = 1 / sum_v fp32_accum(exp(x-1)) = e^{1-Zx}
        f = Ln((1-a)*e + a),  e = Exp(D~ + dz),  D~ = bf16(y - x),
        dz = Zx - Zy,  a = clip(1 - 0.9/(exp((Sx-Sy)/mask_tot)+1e-5), .01, .1)
  loss = (2-BETA)*mean(rowval even) + BETA*mean(rowval odd).

Key wins vs v1 (853us -> target <500us):
  * ACT transcendentals cut 5/elem -> 4/elem: pass-2's p=exp(x-Zx) comes from
    the persistent fp8 P~ tile (125 KB/partition in SBUF) written as the
    pass-1 sumexp's activation output (bias=-1 keeps e^(x-1) <= 134 < 240
    e4m3 max).
  * Inputs are pre-cast to bf16 on host (sharding/transport choice): pass-1
    HBM traffic halves, DMA is plain HWDGE (no SWDGE cast contention).
  * Pass 2 streams only D~ = y - x (bf16, staged to a DRAM bounce buffer in
    pass 1): 32.8 MB/core round trip instead of re-reading x,y (131 MB f32).
  * Pass-2 multiply+reduce fused into one scalar_tensor_tensor with
    accum_out, folding the per-token sct scalar for free.
  * Pass-2's e = Exp(D~ + dz) computed on the otherwise-loaded-down ScalarE
    is replaced by a Schraudolph fast-exp on DVE: one 2x tensor_scalar
    (v = D~*(2^7/ln2) + (dz*S + C)) -> int16 with round-to-nearest, whose
    bit pattern IS bf16(e) (log-mean-centered C; +-4% sawtooth cancels in
    the p-weighted sum; end-to-end 3.2e-4 rel err in simulation). ACT drops
    to 3 transcendental ops/elem.

Sharding: core c handles batch row c//2, token half c%2 (512 of the 1023
valid shifted tokens; slot 1023 masked). The x/y role swap makes the SPMD
graph identical on all 8 cores. Only cross-core exchange: [1,2] AllReduce of
(Sx, Sy) within each core pair.
"""
import os
import numpy as np
import ml_dtypes

import bass_rust as _bass_rust
from concourse import bacc, tile, mybir
from concourse.bass_utils import run_bass_kernel_spmd
from concourse.hw_specs import get_activation_tables


class _OneActSetBacc(bacc.Bacc):
    """Force Exp and Ln to resolve to the single act-function set that
    contains both (``natural_log_exp_and_others``), so alternating Exp/Ln
    activations emit zero ACT_TABLE_LOADs after the first. Entry order is
    preserved so act_func_set_id indices still match act_info.json."""

    def insert_act_table_loads(self):
        has_activation = any(
            isinstance(i, mybir.InstActivation)
            for b in self.main_func.blocks
            for i in b.instructions
        )
        if not has_activation:
            return
        tables = get_activation_tables(self.m.arch)
        both = "natural_log_exp_and_others"
        exp, ln = (
            mybir.ActivationFunctionType.Exp,
            mybir.ActivationFunctionType.Ln,
        )
        if both in tables and {exp, ln} <= tables[both]:
            tables = {
                name: (fns if name == both else fns - {exp, ln})
                for name, fns in tables.items()
            }
        _bass_rust.insert_act_table_loads(self, list(tables.items()))


B, T, V = 4, 1024, 32000
P = 128                 # SBUF partitions = tokens per block
TPC = 512               # token slots per core
NCORES = 8
IGNORE = -100
BASE_ALPHA = 0.1
BETA = 1.0
F32 = mybir.dt.float32
BF16 = mybir.dt.bfloat16
E4 = mybir.dt.float8e4
E3 = mybir.dt.float8e3
I16 = mybir.dt.int16
AX = mybir.AxisListType
ALU = mybir.AluOpType
ACTF = mybir.ActivationFunctionType

REPLICA_GROUPS = [[0, 1], [2, 3], [4, 5], [6, 7]]
SAW_S = 184.6649652337873     # 2^7/ln2
SAW_C = 16256.0 - 7.335       # 127*2^7, log-mean-centered


def build_nc(tpc=TPC, v=V, w=4000, host_cast=True):
    ntb = tpc // P
    nch = v // w
    assert ntb * P == tpc and nch * w == v
    IN_DT = BF16 if host_cast else F32

    nc = _OneActSetBacc(
        "TRN2", target_bir_lowering=False, debug=False, num_devices=NCORES
    )
    x_d = nc.dram_tensor("x", [tpc, v], IN_DT, kind="ExternalInput")
    y_d = nc.dram_tensor("y", [tpc, v], IN_DT, kind="ExternalInput")
    xlab_d = nc.dram_tensor("xlab", [P, ntb], F32, kind="ExternalInput")
    ylab_d = nc.dram_tensor("ylab", [P, ntb], F32, kind="ExternalInput")
    mask_d = nc.dram_tensor("mask", [P, ntb], F32, kind="ExternalInput")
    invm_d = nc.dram_tensor("invm", [1, 1], F32, kind="ExternalInput")
    out_d = nc.dram_tensor("out", [1, 4], F32, kind="ExternalOutput")

    with tile.TileContext(nc) as tc:
        with (
            tc.tile_pool(name="big", bufs=2) as big,
            tc.tile_pool(name="dst", bufs=2) as dstp,
            tc.tile_pool(name="ef", bufs=2) as scr,
            tc.tile_pool(name="i16", bufs=2) as i16p,
            tc.tile_pool(name="blk", bufs=2) as blk,
            tc.tile_pool(name="sm", bufs=1) as sm,
            tc.tile_pool(name="psum", bufs=2, space="PSUM") as psp,
            tc.tile_pool(name="dram", bufs=2, space="DRAM") as dram,
        ):
            # persistent tiles
            ptil = sm.tile([P, ntb * v], E4, tag="ptil")  # e4m3 cache of e^(x-1)
            zx = sm.tile([P, ntb], F32, tag="zx")         # ln sumexp(x-1) = Zx-1
            zy = sm.tile([P, ntb], F32, tag="zy")
            dz = sm.tile([P, ntb], F32, tag="dz")         # Zx - Zy
            sct = sm.tile([P, ntb], F32, tag="sct")       # e^{1-Zx}
            dzs = sm.tile([P, ntb], F32, tag="dzs")       # dz*SAW_S + SAW_C
            axc = sm.tile([P, nch], F32, tag="axc")       # per-chunk sumexp(x-1)
            ayc = sm.tile([P, nch], F32, tag="ayc")
            tac = sm.tile([P, nch], F32, tag="tac")       # per-chunk sct*sum P~ f
            term = sm.tile([P, ntb], F32, tag="term")     # per-token sum_v p*f
            xlab = sm.tile([P, ntb], F32, tag="xlab")     # host: x[t,lbl]-1
            ylab = sm.tile([P, ntb], F32, tag="ylab")
            mask = sm.tile([P, ntb], F32, tag="mask")
            sxsy = sm.tile([P, 2], F32, tag="sxsy")
            ones = sm.tile([P, 1], F32, tag="ones")
            ones_row = sm.tile([1, P], F32, tag="ones_row")
            neg1 = sm.tile([P, 1], F32, tag="neg1")
            invm_sb = sm.tile([1, 1], F32, tag="invm_sb")
            allr = sm.tile([1, 2], F32, tag="allr")       # allreduced (Sx, Sy)
            alpha_b = sm.tile([P, 1], F32, tag="alpha_b")
            oma_b = sm.tile([P, 1], F32, tag="oma_b")
            out_sb = sm.tile([1, 4], F32, tag="out_sb")

            nc.vector.memset(ones[:], 1.0)
            nc.vector.memset(ones_row[:], 1.0)
            nc.vector.memset(neg1[:], -1.0)
            nc.sync.dma_start(out=xlab[:], in_=xlab_d[:])
            nc.sync.dma_start(out=ylab[:], in_=ylab_d[:])
            nc.sync.dma_start(out=mask[:], in_=mask_d[:])
            nc.sync.dma_start(out=invm_sb[:], in_=invm_d[:])

            dt_dram = dram.tile([tpc, v], BF16, tag="dt_dram")  # D~ bounce

            in_dma = nc.sync if host_cast else nc.gpsimd

            # ---------------- pass 1 ----------------
            for tb in range(ntb):
                rs = tb * P
                for c in range(nch):
                    ds_ = c * w
                    xt = big.tile([P, w], BF16, tag="xt")
                    in_dma.dma_start(out=xt[:], in_=x_d[rs:rs + P, ds_:ds_ + w])
                    yt = big.tile([P, w], BF16, tag="yt")
                    in_dma.dma_start(out=yt[:], in_=y_d[rs:rs + P, ds_:ds_ + w])
                    # sumexp(x-1) with the e4m3 prob cache as free output
                    nc.scalar.activation(
                        out=ptil[:, tb * v + ds_: tb * v + ds_ + w],
                        in_=xt[:], func=ACTF.Exp, bias=neg1[:],
                        accum_out=axc[:, c:c + 1],
                    )
                    ey = scr.tile([P, w], BF16, tag="ft")
                    nc.scalar.activation(
                        out=ey[:], in_=yt[:], func=ACTF.Exp, bias=neg1[:],
                        accum_out=ayc[:, c:c + 1],
                    )
                    dt = dstp.tile([P, w], BF16, tag="dt")
                    nc.vector.tensor_sub(dt[:], yt[:], xt[:])
                    nc.sync.dma_start(
                        out=dt_dram[rs:rs + P, ds_:ds_ + w], in_=dt[:]
                    )
                sume_x = blk.tile([P, 1], F32, tag="sume_x")
                nc.vector.reduce_sum(out=sume_x[:], in_=axc[:], axis=AX.X)
                nc.scalar.activation(out=zx[:, tb:tb + 1], in_=sume_x[:], func=ACTF.Ln)
                nc.vector.reciprocal(sct[:, tb:tb + 1], sume_x[:])
                sume_y = blk.tile([P, 1], F32, tag="sume_y")
                nc.vector.reduce_sum(out=sume_y[:], in_=ayc[:], axis=AX.X)
                nc.scalar.activation(out=zy[:, tb:tb + 1], in_=sume_y[:], func=ACTF.Ln)

            nc.vector.tensor_sub(dz[:], zx[:], zy[:])
            nc.vector.tensor_scalar(dzs[:], dz[:], SAW_S, SAW_C, ALU.mult, ALU.add)

            # label partial sums Sx, Sy over this core's tokens
            # (xlab/ylab arrive host-adjusted by -1 to match zx = Zx-1)
            ptx = blk.tile([P, ntb], F32, tag="ptx")
            nc.vector.tensor_sub(ptx[:], xlab[:], zx[:])
            ttx = blk.tile([P, ntb], F32, tag="ttx")
            nc.vector.tensor_mul(ttx[:], ptx[:], mask[:])
            nc.vector.reduce_sum(out=sxsy[:, 0:1], in_=ttx[:], axis=AX.X)
            pty = blk.tile([P, ntb], F32, tag="pty")
            nc.vector.tensor_sub(pty[:], ylab[:], zy[:])
            tty = blk.tile([P, ntb], F32, tag="tty")
            nc.vector.tensor_mul(tty[:], pty[:], mask[:])
            nc.vector.reduce_sum(out=sxsy[:, 1:2], in_=tty[:], axis=AX.X)
            # partition-reduce via matmul with ones: [128,2] -> [1,2]
            ps2 = psp.tile([1, 2], F32, tag="ps2")
            nc.tensor.matmul(ps2[:], ones[:], sxsy[:])
            sb2 = blk.tile([1, 2], F32, tag="sb2")
            nc.vector.tensor_copy(sb2[:], ps2[:])

            in_bounce = dram.tile([1, 2], F32, tag="in_bounce")
            out_bounce = dram.tile([1, 2], F32, tag="out_bounce")
            nc.sync.dma_start(out=in_bounce[:], in_=sb2[:])
            nc.gpsimd.collective_compute(
                "AllReduce", ALU.add, replica_groups=REPLICA_GROUPS,
                ins=[in_bounce[:].opt()], outs=[out_bounce[:].opt()],
            )
            nc.sync.dma_start(out=allr[:], in_=out_bounce[:])

            # alpha = clip(1 - 0.9/(exp((Sx-Sy)*invm) + 1e-5), 0.01, 0.1)
            t1 = blk.tile([1, 1], F32, tag="t1")
            nc.vector.tensor_sub(t1[:], allr[0:1, 0:1], allr[0:1, 1:2])
            t2 = blk.tile([1, 1], F32, tag="t2")
            nc.vector.tensor_mul(t2[:], t1[:], invm_sb[:])
            t3 = blk.tile([1, 1], F32, tag="t3")
            nc.scalar.activation(out=t3[:], in_=t2[:], func=ACTF.Exp)
            t4 = blk.tile([1, 1], F32, tag="t4")
            nc.vector.tensor_scalar_add(t4[:], t3[:], 1e-5)
            t5 = blk.tile([1, 1], F32, tag="t5")
            nc.vector.reciprocal(t5[:], t4[:])
            t6 = blk.tile([1, 1], F32, tag="t6")
            nc.vector.tensor_scalar(
                t6[:], t5[:], -(1.0 - BASE_ALPHA), 1.0, ALU.mult, ALU.add
            )
            al = blk.tile([1, 1], F32, tag="al")
            nc.vector.tensor_scalar(
                al[:], t6[:], BASE_ALPHA, 0.01, ALU.min, ALU.max
            )
            om = blk.tile([1, 1], F32, tag="om")
            nc.vector.tensor_scalar(om[:], al[:], -1.0, 1.0, ALU.mult, ALU.add)
            alom = blk.tile([1, 2], F32, tag="alom")
            nc.vector.tensor_copy(alom[0:1, 0:1], al[:])
            nc.vector.tensor_copy(alom[0:1, 1:2], om[:])
            bc_ps = psp.tile([P, 2], F32, tag="bc_ps")
            nc.tensor.matmul(bc_ps[:], ones_row[:], alom[:])
            nc.vector.tensor_copy(alpha_b[:], bc_ps[:, 0:1])
            nc.vector.tensor_copy(oma_b[:], bc_ps[:, 1:2])

            # ---------------- pass 2 ----------------
            for tb in range(ntb):
                rs = tb * P
                for c in range(nch):
                    ds_ = c * w
                    dti = big.tile([P, w], BF16, tag="yt")
                    nc.sync.dma_start(
                        out=dti[:], in_=dt_dram[rs:rs + P, ds_:ds_ + w]
                    )
                    # Schraudolph fast-exp on DVE: int16 bits are bf16(e)
                    es = i16p.tile([P, w], I16, tag="es")
                    nc.vector.tensor_scalar(
                        es[:], dti[:], SAW_S, dzs[:, tb:tb + 1],
                        ALU.mult, ALU.add,
                    )
                    ft = scr.tile([P, w], BF16, tag="ft")
                    nc.scalar.activation(
                        out=ft[:], in_=es[:].bitcast(BF16), func=ACTF.Ln,
                        bias=alpha_b[:], scale=oma_b[:],
                    )
                    # (P~ * sct) * f, free-dim-summed into tac
                    # (out overwrites the dead dti tile to save SBUF)
                    nc.vector.scalar_tensor_tensor(
                        out=dti[:],
                        in0=ptil[:, tb * v + ds_: tb * v + ds_ + w],
                        scalar=sct[:, tb:tb + 1], in1=ft[:],
                        op0=ALU.mult, op1=ALU.mult,
                        accum_out=tac[:, c:c + 1],
                    )
                nc.vector.reduce_sum(out=term[:, tb:tb + 1], in_=tac[:], axis=AX.X)

            # core partial = sum_t mask * term
            tmr = blk.tile([P, ntb], F32, tag="tmr")
            tmc = blk.tile([P, 1], F32, tag="tmc")
            nc.vector.tensor_mul(tmr[:], term[:], mask[:])
            nc.vector.reduce_sum(out=tmc[:], in_=tmr[:], axis=AX.X)
            ps1 = psp.tile([1, 1], F32, tag="ps1")
            nc.tensor.matmul(ps1[:], ones[:], tmc[:])
            nc.vector.tensor_copy(out_sb[0:1, 0:1], ps1[:])
            nc.vector.tensor_copy(out_sb[0:1, 1:3], allr[:])
            nc.vector.tensor_copy(out_sb[0:1, 3:4], al[:])
            nc.sync.dma_start(out=out_d[:], in_=out_sb[:])

    nc.compile()
    return nc


def host_prepare(student, teacher, labels, host_cast=True):
    """Per-core input maps. Sharding + bf16 transport cast on host."""
    student = np.asarray(student, dtype=np.float32)
    teacher = np.asarray(teacher, dtype=np.float32)
    labels = np.asarray(labels)
    ntb = TPC // P
    in_dt = ml_dtypes.bfloat16 if host_cast else np.float32
    in_maps = []
    invms = []
    for core in range(NCORES):
        r, h = core // 2, core % 2
        if r % 2 == 0:
            x_full, y_full = teacher[r], student[r]
        else:
            x_full, y_full = student[r], teacher[r]
        sl = slice(h * TPC, (h + 1) * TPC)
        x = np.ascontiguousarray(x_full[sl]).astype(in_dt)
        y = np.ascontiguousarray(y_full[sl]).astype(in_dt)
        t_global = h * TPC + np.arange(TPC)
        valid = t_global <= T - 2
        lbl = np.where(valid, labels[r][np.minimum(t_global + 1, T - 1)], 0)
        m = ((lbl != IGNORE) & valid).astype(np.float32)
        lbl_c = np.clip(lbl, 0, V - 1)
        # gather from the device-visible (cast) values; -1 matches zx = Zx-1
        xlab = x[np.arange(TPC), lbl_c].astype(np.float32) - 1.0
        ylab = y[np.arange(TPC), lbl_c].astype(np.float32) - 1.0
        row_lbl = labels[r][1:]
        mask_total = float(np.maximum((row_lbl != IGNORE).sum(), 1.0))
        invms.append(1.0 / mask_total)

        def fold(vec):
            return np.ascontiguousarray(vec.reshape(ntb, P).T.astype(np.float32))

        in_maps.append({
            "x": x,
            "y": y,
            "xlab": fold(xlab),
            "ylab": fold(ylab),
            "mask": fold(m),
            "invm": np.array([[1.0 / mask_total]], dtype=np.float32),
        })
    return in_maps, invms


def host_combine(results, invms):
    partials = [float(results[i]["out"][0, 0]) for i in range(NCORES)]
    row_vals = []
    for r in range(B):
        pA, pB = partials[2 * r], partials[2 * r + 1]
        row_vals.append(-(pA + pB) * invms[2 * r])
    loss = (2.0 - BETA) * (row_vals[0] + row_vals[2]) / 2.0 \
        + BETA * (row_vals[1] + row_vals[3]) / 2.0
    return np.array(loss, dtype=np.float32)


_NC = None
LAST_RESULT = None  # BassKernelResults from the most recent run (for profiling)


def kernel(student_logits=None, teacher_logits=None, labels=None):
    global _NC, LAST_RESULT
    host_cast = os.environ.get("KERNEL_HOST_CAST", "1") == "1"
    if _NC is None:
        _NC = build_nc(
            w=int(os.environ.get("KERNEL_W", "4000")),
            host_cast=host_cast,
        )
    in_maps, invms = host_prepare(
        student_logits, teacher_logits, labels, host_cast=host_cast
    )
    res = run_bass_kernel_spmd(
        _NC, in_maps, core_ids=list(range(NCORES)),
        trace=bool(os.environ.get("KERNEL_TRACE")),
    )
    LAST_RESULT = res
    return host_combine(res.results, invms)


# revision 6
# speedup vs baseline: 1.0828x; 1.0828x over previous
"""Distilled-KL loss head on 8 TRN2 NeuronCores — v4.

Math (validated vs the jax reference, see numsim.py):
  For batch row r, with x = teacher logits (even r) / student (odd r), y the
  other tensor, the per-row term is
      rowval = -(1/mask_tot) * sum_t mask_t * sct_t * sum_v P~_v * f_v
  where P~ = e4m3(exp(x - 1))   [fp8 prob cache, written free by pass-1 exp]
        sct = 1 / sum_v fp32_accum(exp(x-1)) = e^{1-Zx}
        f = Ln((1-a)*e + a),  e ~= exp(D~ + dz),  D~ = bf16(y - x),
        dz = Zx - Zy,  a = clip(1 - 0.9/(exp((Sx-Sy)/mask_tot)+1e-5), .01, .1)
  loss = (2-BETA)*mean(rowval even) + BETA*mean(rowval odd).

Performance structure (853us baseline -> v2 571us -> v4):
  * ACT transcendentals 5/elem -> 3/elem:
      - pass-2 p=exp(x-Zx) comes from the persistent fp8 P~ tile
        (125 KB/partition SBUF) written as the pass-1 sumexp's activation
        output (bias=-1 keeps e^(x-1) <= 134 < 240 e4m3 max);
      - pass-2 e=exp(D~+dz) is a Schraudolph fast-exp on the idle DVE:
        one 4x tensor_scalar (v = D~*(2^7/ln2) + (dz*S+C)) -> int16
        round-to-nearest whose bit pattern IS bf16(e); log-mean-centered C,
        +-4% sawtooth cancels in the p-weighted sum (9.5e-4 end-to-end).
  * Inputs pre-cast to fp8 e3m4 on host (transport/sharding choice):
    pass-1 HBM read is 32.8 MB/core; with the bf16 D~ bounce write
    (32.8 MB round trip) pass 1 stays under the 358 GB/s HBM-per-core cap.
  * Pass 2 streams only D~; mul+reduce fused into one scalar_tensor_tensor
    with f32 accum_out, folding the per-token sct scalar; its dummy output
    overwrites the dead es tile (zero extra SBUF, no WAR hazards).
  * Pass-2 software-pipelined with a one-chunk skew (ts_{k+1} emitted
    before ft_k/stt_k) so DVE streams at stt+ts with no ACT ping-pong.
  * Dummy warmup AllReduce at kernel start pre-pays collective ring setup,
    shrinking the pass-boundary alpha bubble.

Sharding: core c handles batch row c//2, token half c%2 (512 of the 1023
valid shifted tokens; slot 1023 masked). The x/y role swap makes the SPMD
graph identical on all 8 cores. Only cross-core exchange: [1,2] AllReduce of
(Sx, Sy) within each core pair.
"""
import os
import numpy as np
import ml_dtypes

import bass_rust as _bass_rust
from concourse import bacc, tile, mybir
from concourse.bass_utils import run_bass_kernel_spmd
from concourse.hw_specs import get_activation_tables


class _OneActSetBacc(bacc.Bacc):
    """Force Exp and Ln to resolve to the single act-function set that
    contains both (``natural_log_exp_and_others``), so alternating Exp/Ln
    activations emit zero ACT_TABLE_LOADs after the first. Entry order is
    preserved so act_func_set_id indices still match act_info.json."""

    def insert_act_table_loads(self):
        has_activation = any(
            isinstance(i, mybir.InstActivation)
            for b in self.main_func.blocks
            for i in b.instructions
        )
        if not has_activation:
            return
        tables = get_activation_tables(self.m.arch)
        both = "natural_log_exp_and_others"
        exp, ln = (
            mybir.ActivationFunctionType.Exp,
            mybir.ActivationFunctionType.Ln,
        )
        if both in tables and {exp, ln} <= tables[both]:
            tables = {
                name: (fns if name == both else fns - {exp, ln})
                for name, fns in tables.items()
            }
        _bass_rust.insert_act_table_loads(self, list(tables.items()))


B, T, V = 4, 1024, 32000
P = 128                 # SBUF partitions = tokens per block
TPC = 512               # token slots per core
NCORES = 8
IGNORE = -100
BASE_ALPHA = 0.1
BETA = 1.0
F32 = mybir.dt.float32
BF16 = mybir.dt.bfloat16
E4 = mybir.dt.float8e4
E3 = mybir.dt.float8e3
I16 = mybir.dt.int16
AX = mybir.AxisListType
ALU = mybir.AluOpType
ACTF = mybir.ActivationFunctionType

REPLICA_GROUPS = [[0, 1], [2, 3], [4, 5], [6, 7]]
SAW_S = 184.6649652337873     # 2^7/ln2
SAW_C = 16256.0 - 7.335       # 127*2^7, log-mean-centered


def build_nc(tpc=TPC, v=V, in_dt="e3", warmup_cc=True):
    IN_DT = {"e3": E3, "bf16": BF16, "f32": F32}[in_dt]
    TILE_DT = BF16 if in_dt == "f32" else IN_DT  # f32 mode: SWDGE cast to bf16
    w = 4000 if in_dt == "e3" else 2000
    ntb = tpc // P
    nch = v // w
    assert ntb * P == tpc and nch * w == v

    nc = _OneActSetBacc(
        "TRN2", target_bir_lowering=False, debug=False, num_devices=NCORES
    )
    x_d = nc.dram_tensor("x", [tpc, v], IN_DT, kind="ExternalInput")
    y_d = nc.dram_tensor("y", [tpc, v], IN_DT, kind="ExternalInput")
    xlab_d = nc.dram_tensor("xlab", [P, ntb], F32, kind="ExternalInput")
    ylab_d = nc.dram_tensor("ylab", [P, ntb], F32, kind="ExternalInput")
    mask_d = nc.dram_tensor("mask", [P, ntb], F32, kind="ExternalInput")
    invm_d = nc.dram_tensor("invm", [1, 1], F32, kind="ExternalInput")
    out_d = nc.dram_tensor("out", [1, 4], F32, kind="ExternalOutput")

    with tile.TileContext(nc) as tc:
        with (
            tc.tile_pool(name="big", bufs=2) as big,
            tc.tile_pool(name="dst", bufs=2) as dstp,
            tc.tile_pool(name="dtip", bufs=2) as dtip,
            tc.tile_pool(name="ef", bufs=2) as scr,
            tc.tile_pool(name="i16", bufs=2) as i16p,
            tc.tile_pool(name="blk", bufs=2) as blk,
            tc.tile_pool(name="sm", bufs=1) as sm,
            tc.tile_pool(name="psum", bufs=2, space="PSUM") as psp,
            tc.tile_pool(name="dram", bufs=2, space="DRAM") as dram,
        ):
            # persistent tiles
            ptil = sm.tile([P, ntb * v], E4, tag="ptil")  # e4m3 cache of e^(x-1)
            zx = sm.tile([P, ntb], F32, tag="zx")         # ln sumexp(x-1) = Zx-1
            zy = sm.tile([P, ntb], F32, tag="zy")
            dz = sm.tile([P, ntb], F32, tag="dz")         # Zx - Zy
            dzs = sm.tile([P, ntb], F32, tag="dzs")       # dz*SAW_S + SAW_C
            sct = sm.tile([P, ntb], F32, tag="sct")       # e^{1-Zx}
            axc = sm.tile([P, nch], F32, tag="axc")       # per-chunk sumexp(x-1)
            ayc = sm.tile([P, nch], F32, tag="ayc")
            tac = sm.tile([P, nch], F32, tag="tac")       # per-chunk sct*sum P~ f
            term = sm.tile([P, ntb], F32, tag="term")     # per-token sum_v p*f
            xlab = sm.tile([P, ntb], F32, tag="xlab")     # host: x[t,lbl]-1
            ylab = sm.tile([P, ntb], F32, tag="ylab")
            mask = sm.tile([P, ntb], F32, tag="mask")
            sxsy = sm.tile([P, 2], F32, tag="sxsy")
            ones = sm.tile([P, 1], F32, tag="ones")
            ones_row = sm.tile([1, P], F32, tag="ones_row")
            neg1 = sm.tile([P, 1], F32, tag="neg1")
            invm_sb = sm.tile([1, 1], F32, tag="invm_sb")
            allr = sm.tile([1, 2], F32, tag="allr")       # allreduced (Sx, Sy)
            alpha_b = sm.tile([P, 1], F32, tag="alpha_b")
            oma_b = sm.tile([P, 1], F32, tag="oma_b")
            out_sb = sm.tile([1, 4], F32, tag="out_sb")

            nc.vector.memset(ones[:], 1.0)
            nc.vector.memset(ones_row[:], 1.0)
            nc.vector.memset(neg1[:], -1.0)
            nc.sync.dma_start(out=xlab[:], in_=xlab_d[:])
            nc.sync.dma_start(out=ylab[:], in_=ylab_d[:])
            nc.sync.dma_start(out=mask[:], in_=mask_d[:])
            nc.sync.dma_start(out=invm_sb[:], in_=invm_d[:])

            if warmup_cc:
                # dummy AllReduce to pre-pay collective ring setup; runs
                # concurrently with early pass-1 compute
                wsb = sm.tile([1, 1], F32, tag="wsb")
                wjk = sm.tile([1, 1], F32, tag="wjk")
                nc.vector.memset(wsb[:], 0.0)
                w_in = dram.tile([1, 1], F32, tag="w_in")
                w_out = dram.tile([1, 1], F32, tag="w_out")
                nc.sync.dma_start(out=w_in[:], in_=wsb[:])
                nc.gpsimd.collective_compute(
                    "AllReduce", ALU.add, replica_groups=REPLICA_GROUPS,
                    ins=[w_in[:].opt()], outs=[w_out[:].opt()],
                )
                nc.sync.dma_start(out=wjk[:], in_=w_out[:])

            dt_dram = dram.tile([tpc, v], BF16, tag="dt_dram")  # D~ bounce

            in_dma = nc.gpsimd if in_dt == "f32" else nc.sync

            # ---------------- pass 1 ----------------
            for tb in range(ntb):
                rs = tb * P
                for c in range(nch):
                    ds_ = c * w
                    xt = big.tile([P, w], TILE_DT, tag="xt")
                    in_dma.dma_start(out=xt[:], in_=x_d[rs:rs + P, ds_:ds_ + w])
                    yt = big.tile([P, w], TILE_DT, tag="yt")
                    in_dma.dma_start(out=yt[:], in_=y_d[rs:rs + P, ds_:ds_ + w])
                    # sumexp(x-1) with the e4m3 prob cache as free output
                    nc.scalar.activation(
                        out=ptil[:, tb * v + ds_: tb * v + ds_ + w],
                        in_=xt[:], func=ACTF.Exp, bias=neg1[:],
                        accum_out=axc[:, c:c + 1],
                    )
                    ey = scr.tile([P, w], BF16, tag="ft")
                    nc.scalar.activation(
                        out=ey[:], in_=yt[:], func=ACTF.Exp, bias=neg1[:],
                        accum_out=ayc[:, c:c + 1],
                    )
                    dt = dstp.tile([P, w], BF16, tag="dt")
                    nc.vector.tensor_sub(dt[:], yt[:], xt[:])
                    nc.sync.dma_start(
                        out=dt_dram[rs:rs + P, ds_:ds_ + w], in_=dt[:]
                    )
                sume_x = blk.tile([P, 1], F32, tag="sume_x")
                nc.vector.reduce_sum(out=sume_x[:], in_=axc[:], axis=AX.X)
                nc.scalar.activation(out=zx[:, tb:tb + 1], in_=sume_x[:], func=ACTF.Ln)
                nc.vector.reciprocal(sct[:, tb:tb + 1], sume_x[:])
                sume_y = blk.tile([P, 1], F32, tag="sume_y")
                nc.vector.reduce_sum(out=sume_y[:], in_=ayc[:], axis=AX.X)
                nc.scalar.activation(out=zy[:, tb:tb + 1], in_=sume_y[:], func=ACTF.Ln)

            nc.vector.tensor_sub(dz[:], zx[:], zy[:])
            nc.vector.tensor_scalar(dzs[:], dz[:], SAW_S, SAW_C, ALU.mult, ALU.add)

            # label partial sums Sx, Sy over this core's tokens
            # (xlab/ylab arrive host-adjusted by -1 to match zx = Zx-1)
            ptx = blk.tile([P, ntb], F32, tag="ptx")
            nc.vector.tensor_sub(ptx[:], xlab[:], zx[:])
            ttx = blk.tile([P, ntb], F32, tag="ttx")
            nc.vector.tensor_mul(ttx[:], ptx[:], mask[:])
            nc.vector.reduce_sum(out=sxsy[:, 0:1], in_=ttx[:], axis=AX.X)
            pty = blk.tile([P, ntb], F32, tag="pty")
            nc.vector.tensor_sub(pty[:], ylab[:], zy[:])
            tty = blk.tile([P, ntb], F32, tag="tty")
            nc.vector.tensor_mul(tty[:], pty[:], mask[:])
            nc.vector.reduce_sum(out=sxsy[:, 1:2], in_=tty[:], axis=AX.X)
            # partition-reduce via matmul with ones: [128,2] -> [1,2]
            ps2 = psp.tile([1, 2], F32, tag="ps2")
            nc.tensor.matmul(ps2[:], ones[:], sxsy[:])
            sb2 = blk.tile([1, 2], F32, tag="sb2")
            nc.vector.tensor_copy(sb2[:], ps2[:])

            in_bounce = dram.tile([1, 2], F32, tag="in_bounce")
            out_bounce = dram.tile([1, 2], F32, tag="out_bounce")
            nc.sync.dma_start(out=in_bounce[:], in_=sb2[:])
            nc.gpsimd.collective_compute(
                "AllReduce", ALU.add, replica_groups=REPLICA_GROUPS,
                ins=[in_bounce[:].opt()], outs=[out_bounce[:].opt()],
            )
            nc.sync.dma_start(out=allr[:], in_=out_bounce[:])

            # alpha = clip(1 - 0.9/(exp((Sx-Sy)*invm) + 1e-5), 0.01, 0.1)
            t1 = blk.tile([1, 1], F32, tag="t1")
            nc.vector.tensor_sub(t1[:], allr[0:1, 0:1], allr[0:1, 1:2])
            t2 = blk.tile([1, 1], F32, tag="t2")
            nc.vector.tensor_mul(t2[:], t1[:], invm_sb[:])
            t3 = blk.tile([1, 1], F32, tag="t3")
            nc.scalar.activation(out=t3[:], in_=t2[:], func=ACTF.Exp)
            t4 = blk.tile([1, 1], F32, tag="t4")
            nc.vector.tensor_scalar_add(t4[:], t3[:], 1e-5)
            t5 = blk.tile([1, 1], F32, tag="t5")
            nc.vector.reciprocal(t5[:], t4[:])
            t6 = blk.tile([1, 1], F32, tag="t6")
            nc.vector.tensor_scalar(
                t6[:], t5[:], -(1.0 - BASE_ALPHA), 1.0, ALU.mult, ALU.add
            )
            al = blk.tile([1, 1], F32, tag="al")
            nc.vector.tensor_scalar(
                al[:], t6[:], BASE_ALPHA, 0.01, ALU.min, ALU.max
            )
            om = blk.tile([1, 1], F32, tag="om")
            nc.vector.tensor_scalar(om[:], al[:], -1.0, 1.0, ALU.mult, ALU.add)
            alom = blk.tile([1, 2], F32, tag="alom")
            nc.vector.tensor_copy(alom[0:1, 0:1], al[:])
            nc.vector.tensor_copy(alom[0:1, 1:2], om[:])
            bc_ps = psp.tile([P, 2], F32, tag="bc_ps")
            nc.tensor.matmul(bc_ps[:], ones_row[:], alom[:])
            nc.vector.tensor_copy(alpha_b[:], bc_ps[:, 0:1])
            nc.vector.tensor_copy(oma_b[:], bc_ps[:, 1:2])

            # ---------------- pass 2 (software-pipelined, skew 1) ----------
            steps = ntb * nch
            front = {}

            def p2_front(k):
                tb, c = divmod(k, nch)
                rs, ds_ = tb * P, c * w
                dti = dtip.tile([P, w], BF16, tag="dti")
                nc.sync.dma_start(
                    out=dti[:], in_=dt_dram[rs:rs + P, ds_:ds_ + w]
                )
                # Schraudolph fast-exp: int16 bits are bf16(e^(D~+dz))
                es = i16p.tile([P, w], I16, tag="es")
                nc.vector.tensor_scalar(
                    es[:], dti[:], SAW_S, dzs[:, tb:tb + 1],
                    ALU.mult, ALU.add,
                )
                front[k] = es

            def p2_back(k):
                tb, c = divmod(k, nch)
                es = front.pop(k)
                ft = scr.tile([P, w], BF16, tag="ft")
                nc.scalar.activation(
                    out=ft[:], in_=es[:].bitcast(BF16), func=ACTF.Ln,
                    bias=alpha_b[:], scale=oma_b[:],
                )
                # (P~ * sct) * f, free-dim-summed into tac; the dummy
                # output overwrites the dead es tile (accum is f32-internal)
                nc.vector.scalar_tensor_tensor(
                    out=es[:],
                    in0=ptil[:, (tb * v + c * w): (tb * v + c * w + w)],
                    scalar=sct[:, tb:tb + 1], in1=ft[:],
                    op0=ALU.mult, op1=ALU.mult,
                    accum_out=tac[:, c:c + 1],
                )
                if c == nch - 1:
                    nc.vector.reduce_sum(
                        out=term[:, tb:tb + 1], in_=tac[:], axis=AX.X
                    )

            for k in range(steps + 1):
                if k < steps:
                    p2_front(k)
                if k >= 1:
                    p2_back(k - 1)

            # core partial = sum_t mask * term
            tmr = blk.tile([P, ntb], F32, tag="tmr")
            tmc = blk.tile([P, 1], F32, tag="tmc")
            nc.vector.tensor_mul(tmr[:], term[:], mask[:])
            nc.vector.reduce_sum(out=tmc[:], in_=tmr[:], axis=AX.X)
            ps1 = psp.tile([1, 1], F32, tag="ps1")
            nc.tensor.matmul(ps1[:], ones[:], tmc[:])
            nc.vector.tensor_copy(out_sb[0:1, 0:1], ps1[:])
            nc.vector.tensor_copy(out_sb[0:1, 1:3], allr[:])
            nc.vector.tensor_copy(out_sb[0:1, 3:4], al[:])
            nc.sync.dma_start(out=out_d[:], in_=out_sb[:])

    nc.compile()
    return nc


def host_prepare(student, teacher, labels, in_dt="e3"):
    """Per-core input maps. Sharding + fp8/bf16 transport cast on host."""
    student = np.asarray(student, dtype=np.float32)
    teacher = np.asarray(teacher, dtype=np.float32)
    labels = np.asarray(labels)
    ntb = TPC // P
    np_dt = {"e3": ml_dtypes.float8_e3m4, "bf16": ml_dtypes.bfloat16,
             "f32": np.float32}[in_dt]
    in_maps = []
    invms = []
    for core in range(NCORES):
        r, h = core // 2, core % 2
        if r % 2 == 0:
            x_full, y_full = teacher[r], student[r]
        else:
            x_full, y_full = student[r], teacher[r]
        sl = slice(h * TPC, (h + 1) * TPC)
        x = np.ascontiguousarray(x_full[sl]).astype(np_dt)
        y = np.ascontiguousarray(y_full[sl]).astype(np_dt)
        t_global = h * TPC + np.arange(TPC)
        valid = t_global <= T - 2
        lbl = np.where(valid, labels[r][np.minimum(t_global + 1, T - 1)], 0)
        m = ((lbl != IGNORE) & valid).astype(np.float32)
        lbl_c = np.clip(lbl, 0, V - 1)
        # gather from the device-visible (cast) values; -1 matches zx = Zx-1
        xlab = x[np.arange(TPC), lbl_c].astype(np.float32) - 1.0
        ylab = y[np.arange(TPC), lbl_c].astype(np.float32) - 1.0
        row_lbl = labels[r][1:]
        mask_total = float(np.maximum((row_lbl != IGNORE).sum(), 1.0))
        invms.append(1.0 / mask_total)

        def fold(vec):
            return np.ascontiguousarray(vec.reshape(ntb, P).T.astype(np.float32))

        in_maps.append({
            "x": x,
            "y": y,
            "xlab": fold(xlab),
            "ylab": fold(ylab),
            "mask": fold(m),
            "invm": np.array([[1.0 / mask_total]], dtype=np.float32),
        })
    return in_maps, invms


def host_combine(results, invms):
    partials = [float(results[i]["out"][0, 0]) for i in range(NCORES)]
    row_vals = []
    for r in range(B):
        pA, pB = partials[2 * r], partials[2 * r + 1]
        row_vals.append(-(pA + pB) * invms[2 * r])
    loss = (2.0 - BETA) * (row_vals[0] + row_vals[2]) / 2.0 \
        + BETA * (row_vals[1] + row_vals[3]) / 2.0
    return np.array(loss, dtype=np.float32)


_NC = None
LAST_RESULT = None  # BassKernelResults from the most recent run (for profiling)


def kernel(student_logits=None, teacher_logits=None, labels=None):
    global _NC, LAST_RESULT
    in_dt = os.environ.get("KERNEL_IN_DT", "e3")
    if _NC is None:
        _NC = build_nc(
            in_dt=in_dt,
            warmup_cc=os.environ.get("KERNEL_WARMUP_CC", "1") == "1",
        )
    in_maps, invms = host_prepare(
        student_logits, teacher_logits, labels, in_dt=in_dt
    )
    res = run_bass_kernel_spmd(
        _NC, in_maps, core_ids=list(range(NCORES)),
        trace=bool(os.environ.get("KERNEL_TRACE")),
    )
    LAST_RESULT = res
    return host_combine(res.results, invms)


# revision 7
# speedup vs baseline: 1.1558x; 1.0674x over previous
"""Distilled-KL loss head on 8 TRN2 NeuronCores — v5.

Math (validated vs the jax reference, see numsim.py):
  For batch row r, with x = teacher logits (even r) / student (odd r), y the
  other tensor, the per-row term is
      rowval = -(1/mask_tot) * sum_t mask_t * sct_t * sum_v P~_v * f_v
  where P~ = e4m3(exp(x - 1))   [fp8 prob cache, written free by pass-1 exp]
        sct = 1 / sum_v fp32_accum(exp(x-1)) = e^{1-Zx}
        f = Ln((1-a)*e + a),  e ~= exp(D~ + dz),  D~ = bf16(y - x),
        dz = Zx - Zy,  a = clip(1 - 0.9/(exp((Sx-Sy)/mask_tot)+1e-5), .01, .1)
  loss = (2-BETA)*mean(rowval even) + BETA*mean(rowval odd).

Performance structure (853us baseline -> v2 571us -> v4):
  * ACT transcendentals 5/elem -> 3/elem:
      - pass-2 p=exp(x-Zx) comes from the persistent fp8 P~ tile
        (125 KB/partition SBUF) written as the pass-1 sumexp's activation
        output (bias=-1 keeps e^(x-1) <= 134 < 240 e4m3 max);
      - pass-2 e=exp(D~+dz) is a Schraudolph fast-exp on the idle DVE:
        one 4x tensor_scalar (v = D~*(2^7/ln2) + (dz*S+C)) -> int16
        round-to-nearest whose bit pattern IS bf16(e); log-mean-centered C,
        +-4% sawtooth cancels in the p-weighted sum (9.5e-4 end-to-end).
  * Inputs pre-cast to fp8 e3m4 on host (transport/sharding choice):
    pass-1 HBM read is 32.8 MB/core; with the bf16 D~ bounce write
    (32.8 MB round trip) pass 1 stays under the 358 GB/s HBM-per-core cap.
  * The sawtooth runs in PASS 1 right after the sub (dz folded instead
    into the Ln's per-token scale: ft = Ln(sc_t*e0 + a), sc_t =
    (1-a)*e^{dz_t}), so the DRAM bounce carries finished sawtooth bits and
    pass-2 DVE is stt-only (no ACT/DVE ping-pong).
  * Pass-1 input DMAs are 2 MB (8000-wide fp8 tiles, two 4000 compute
    slices) to stay on the fast side of the DMA-efficiency knee.
  * mul+reduce fused into one scalar_tensor_tensor with f32 accum_out,
    folding the per-token sct scalar; its dummy output overwrites the dead
    es tile (zero extra SBUF, no WAR hazards).
  * Dummy warmup AllReduce at kernel start pre-pays collective ring setup,
    shrinking the pass-boundary alpha bubble.

Sharding: core c handles batch row c//2, token half c%2 (512 of the 1023
valid shifted tokens; slot 1023 masked). The x/y role swap makes the SPMD
graph identical on all 8 cores. Only cross-core exchange: [1,2] AllReduce of
(Sx, Sy) within each core pair.
"""
import os
import numpy as np
import ml_dtypes

import bass_rust as _bass_rust
from concourse import bacc, tile, mybir
from concourse.bass_utils import run_bass_kernel_spmd
from concourse.hw_specs import get_activation_tables


class _OneActSetBacc(bacc.Bacc):
    """Force Exp and Ln to resolve to the single act-function set that
    contains both (``natural_log_exp_and_others``), so alternating Exp/Ln
    activations emit zero ACT_TABLE_LOADs after the first. Entry order is
    preserved so act_func_set_id indices still match act_info.json."""

    def insert_act_table_loads(self):
        has_activation = any(
            isinstance(i, mybir.InstActivation)
            for b in self.main_func.blocks
            for i in b.instructions
        )
        if not has_activation:
            return
        tables = get_activation_tables(self.m.arch)
        both = "natural_log_exp_and_others"
        exp, ln = (
            mybir.ActivationFunctionType.Exp,
            mybir.ActivationFunctionType.Ln,
        )
        if both in tables and {exp, ln} <= tables[both]:
            tables = {
                name: (fns if name == both else fns - {exp, ln})
                for name, fns in tables.items()
            }
        _bass_rust.insert_act_table_loads(self, list(tables.items()))


B, T, V = 4, 1024, 32000
P = 128                 # SBUF partitions = tokens per block
TPC = 512               # token slots per core
NCORES = 8
IGNORE = -100
BASE_ALPHA = 0.1
BETA = 1.0
F32 = mybir.dt.float32
BF16 = mybir.dt.bfloat16
E4 = mybir.dt.float8e4
E3 = mybir.dt.float8e3
I16 = mybir.dt.int16
AX = mybir.AxisListType
ALU = mybir.AluOpType
ACTF = mybir.ActivationFunctionType

REPLICA_GROUPS = [[0, 1], [2, 3], [4, 5], [6, 7]]
SAW_S = 184.6649652337873     # 2^7/ln2
SAW_C = 16256.0 - 7.335       # 127*2^7, log-mean-centered


def build_nc(tpc=TPC, v=V, in_dt="e3", warmup_cc=True):
    IN_DT = {"e3": E3, "bf16": BF16, "f32": F32}[in_dt]
    TILE_DT = BF16 if in_dt == "f32" else IN_DT  # f32 mode: SWDGE cast to bf16
    w = 4000 if in_dt == "e3" else 2000
    w_dma = 2 * w if in_dt == "e3" else w   # 2 MB input DMAs in fp8 mode
    spc = w_dma // w
    ntb = tpc // P
    nch = v // w
    ndc = v // w_dma
    assert ntb * P == tpc and nch * w == v and ndc * w_dma == v

    nc = _OneActSetBacc(
        "TRN2", target_bir_lowering=False, debug=False, num_devices=NCORES
    )
    x_d = nc.dram_tensor("x", [tpc, v], IN_DT, kind="ExternalInput")
    y_d = nc.dram_tensor("y", [tpc, v], IN_DT, kind="ExternalInput")
    xlab_d = nc.dram_tensor("xlab", [P, ntb], F32, kind="ExternalInput")
    ylab_d = nc.dram_tensor("ylab", [P, ntb], F32, kind="ExternalInput")
    mask_d = nc.dram_tensor("mask", [P, ntb], F32, kind="ExternalInput")
    invm_d = nc.dram_tensor("invm", [1, 1], F32, kind="ExternalInput")
    out_d = nc.dram_tensor("out", [1, 4], F32, kind="ExternalOutput")

    with tile.TileContext(nc) as tc:
        with (
            tc.tile_pool(name="big", bufs=2) as big,
            tc.tile_pool(name="dst", bufs=2) as dstp,
            tc.tile_pool(name="ef", bufs=2) as scr,
            tc.tile_pool(name="i16", bufs=2) as i16p,
            tc.tile_pool(name="blk", bufs=2) as blk,
            tc.tile_pool(name="sm", bufs=1) as sm,
            tc.tile_pool(name="psum", bufs=2, space="PSUM") as psp,
            tc.tile_pool(name="dram", bufs=2, space="DRAM") as dram,
        ):
            # persistent tiles
            ptil = sm.tile([P, ntb * v], E4, tag="ptil")  # e4m3 cache of e^(x-1)
            zx = sm.tile([P, ntb], F32, tag="zx")         # ln sumexp(x-1) = Zx-1
            zy = sm.tile([P, ntb], F32, tag="zy")
            dz = sm.tile([P, ntb], F32, tag="dz")         # Zx - Zy
            edz = sm.tile([P, ntb], F32, tag="edz")       # e^{dz}
            scb = sm.tile([P, ntb], F32, tag="scb")       # (1-a)*e^{dz}
            sct = sm.tile([P, ntb], F32, tag="sct")       # e^{1-Zx}
            axc = sm.tile([P, nch], F32, tag="axc")       # per-chunk sumexp(x-1)
            ayc = sm.tile([P, nch], F32, tag="ayc")
            tac = sm.tile([P, nch], F32, tag="tac")       # per-chunk sct*sum P~ f
            term = sm.tile([P, ntb], F32, tag="term")     # per-token sum_v p*f
            xlab = sm.tile([P, ntb], F32, tag="xlab")     # host: x[t,lbl]-1
            ylab = sm.tile([P, ntb], F32, tag="ylab")
            mask = sm.tile([P, ntb], F32, tag="mask")
            sxsy = sm.tile([P, 2], F32, tag="sxsy")
            ones = sm.tile([P, 1], F32, tag="ones")
            ones_row = sm.tile([1, P], F32, tag="ones_row")
            neg1 = sm.tile([P, 1], F32, tag="neg1")
            invm_sb = sm.tile([1, 1], F32, tag="invm_sb")
            allr = sm.tile([1, 2], F32, tag="allr")       # allreduced (Sx, Sy)
            alpha_b = sm.tile([P, 1], F32, tag="alpha_b")
            oma_b = sm.tile([P, 1], F32, tag="oma_b")
            out_sb = sm.tile([1, 4], F32, tag="out_sb")

            nc.vector.memset(ones[:], 1.0)
            nc.vector.memset(ones_row[:], 1.0)
            nc.vector.memset(neg1[:], -1.0)
            nc.sync.dma_start(out=xlab[:], in_=xlab_d[:])
            nc.sync.dma_start(out=ylab[:], in_=ylab_d[:])
            nc.sync.dma_start(out=mask[:], in_=mask_d[:])
            nc.sync.dma_start(out=invm_sb[:], in_=invm_d[:])

            if warmup_cc:
                # dummy AllReduce to pre-pay collective ring setup; runs
                # concurrently with early pass-1 compute
                wsb = sm.tile([1, 1], F32, tag="wsb")
                wjk = sm.tile([1, 1], F32, tag="wjk")
                nc.vector.memset(wsb[:], 0.0)
                w_in = dram.tile([1, 1], F32, tag="w_in")
                w_out = dram.tile([1, 1], F32, tag="w_out")
                nc.sync.dma_start(out=w_in[:], in_=wsb[:])
                nc.gpsimd.collective_compute(
                    "AllReduce", ALU.add, replica_groups=REPLICA_GROUPS,
                    ins=[w_in[:].opt()], outs=[w_out[:].opt()],
                )
                nc.sync.dma_start(out=wjk[:], in_=w_out[:])

            es_dram = dram.tile([tpc, v], I16, tag="es_dram")  # sawtooth-bits bounce

            in_dma = nc.gpsimd if in_dt == "f32" else nc.sync

            # ---------------- pass 1 ----------------
            for tb in range(ntb):
                rs = tb * P
                for cd in range(ndc):
                    dd = cd * w_dma
                    xt = big.tile([P, w_dma], TILE_DT, tag="xt")
                    in_dma.dma_start(out=xt[:], in_=x_d[rs:rs + P, dd:dd + w_dma])
                    yt = big.tile([P, w_dma], TILE_DT, tag="yt")
                    in_dma.dma_start(out=yt[:], in_=y_d[rs:rs + P, dd:dd + w_dma])
                    for sp in range(spc):
                        c = cd * spc + sp
                        ds_ = c * w
                        sl = slice(sp * w, (sp + 1) * w)
                        # sumexp(x-1) with the e4m3 prob cache as free output
                        nc.scalar.activation(
                            out=ptil[:, tb * v + ds_: tb * v + ds_ + w],
                            in_=xt[:, sl], func=ACTF.Exp, bias=neg1[:],
                            accum_out=axc[:, c:c + 1],
                        )
                        ey = dstp.tile([P, w], BF16, tag="dt")
                        nc.scalar.activation(
                            out=ey[:], in_=yt[:, sl], func=ACTF.Exp, bias=neg1[:],
                            accum_out=ayc[:, c:c + 1],
                        )
                        dt = dstp.tile([P, w], BF16, tag="dt")
                        nc.vector.tensor_sub(dt[:], yt[:, sl], xt[:, sl])
                        # sawtooth bits (dz applied later via the Ln scale)
                        es = i16p.tile([P, w], I16, tag="es")
                        nc.vector.tensor_scalar(
                            es[:], dt[:], SAW_S, SAW_C, ALU.mult, ALU.add,
                        )
                        nc.sync.dma_start(
                            out=es_dram[rs:rs + P, ds_:ds_ + w], in_=es[:]
                        )
                sume_x = blk.tile([P, 1], F32, tag="sume_x")
                nc.vector.reduce_sum(out=sume_x[:], in_=axc[:], axis=AX.X)
                nc.scalar.activation(out=zx[:, tb:tb + 1], in_=sume_x[:], func=ACTF.Ln)
                nc.vector.reciprocal(sct[:, tb:tb + 1], sume_x[:])
                sume_y = blk.tile([P, 1], F32, tag="sume_y")
                nc.vector.reduce_sum(out=sume_y[:], in_=ayc[:], axis=AX.X)
                nc.scalar.activation(out=zy[:, tb:tb + 1], in_=sume_y[:], func=ACTF.Ln)

            nc.vector.tensor_sub(dz[:], zx[:], zy[:])
            nc.scalar.activation(out=edz[:], in_=dz[:], func=ACTF.Exp)

            # label partial sums Sx, Sy over this core's tokens
            # (xlab/ylab arrive host-adjusted by -1 to match zx = Zx-1)
            ptx = blk.tile([P, ntb], F32, tag="ptx")
            nc.vector.tensor_sub(ptx[:], xlab[:], zx[:])
            ttx = blk.tile([P, ntb], F32, tag="ttx")
            nc.vector.tensor_mul(ttx[:], ptx[:], mask[:])
            nc.vector.reduce_sum(out=sxsy[:, 0:1], in_=ttx[:], axis=AX.X)
            pty = blk.tile([P, ntb], F32, tag="pty")
            nc.vector.tensor_sub(pty[:], ylab[:], zy[:])
            tty = blk.tile([P, ntb], F32, tag="tty")
            nc.vector.tensor_mul(tty[:], pty[:], mask[:])
            nc.vector.reduce_sum(out=sxsy[:, 1:2], in_=tty[:], axis=AX.X)
            # partition-reduce via matmul with ones: [128,2] -> [1,2]
            ps2 = psp.tile([1, 2], F32, tag="ps2")
            nc.tensor.matmul(ps2[:], ones[:], sxsy[:])
            sb2 = blk.tile([1, 2], F32, tag="sb2")
            nc.vector.tensor_copy(sb2[:], ps2[:])

            in_bounce = dram.tile([1, 2], F32, tag="in_bounce")
            out_bounce = dram.tile([1, 2], F32, tag="out_bounce")
            nc.sync.dma_start(out=in_bounce[:], in_=sb2[:])
            nc.gpsimd.collective_compute(
                "AllReduce", ALU.add, replica_groups=REPLICA_GROUPS,
                ins=[in_bounce[:].opt()], outs=[out_bounce[:].opt()],
            )
            nc.sync.dma_start(out=allr[:], in_=out_bounce[:])

            # alpha = clip(1 - 0.9/(exp((Sx-Sy)*invm) + 1e-5), 0.01, 0.1)
            t1 = blk.tile([1, 1], F32, tag="t1")
            nc.vector.tensor_sub(t1[:], allr[0:1, 0:1], allr[0:1, 1:2])
            t2 = blk.tile([1, 1], F32, tag="t2")
            nc.vector.tensor_mul(t2[:], t1[:], invm_sb[:])
            t3 = blk.tile([1, 1], F32, tag="t3")
            nc.scalar.activation(out=t3[:], in_=t2[:], func=ACTF.Exp)
            t4 = blk.tile([1, 1], F32, tag="t4")
            nc.vector.tensor_scalar_add(t4[:], t3[:], 1e-5)
            t5 = blk.tile([1, 1], F32, tag="t5")
            nc.vector.reciprocal(t5[:], t4[:])
            t6 = blk.tile([1, 1], F32, tag="t6")
            nc.vector.tensor_scalar(
                t6[:], t5[:], -(1.0 - BASE_ALPHA), 1.0, ALU.mult, ALU.add
            )
            al = blk.tile([1, 1], F32, tag="al")
            nc.vector.tensor_scalar(
                al[:], t6[:], BASE_ALPHA, 0.01, ALU.min, ALU.max
            )
            om = blk.tile([1, 1], F32, tag="om")
            nc.vector.tensor_scalar(om[:], al[:], -1.0, 1.0, ALU.mult, ALU.add)
            alom = blk.tile([1, 2], F32, tag="alom")
            nc.vector.tensor_copy(alom[0:1, 0:1], al[:])
            nc.vector.tensor_copy(alom[0:1, 1:2], om[:])
            bc_ps = psp.tile([P, 2], F32, tag="bc_ps")
            nc.tensor.matmul(bc_ps[:], ones_row[:], alom[:])
            nc.vector.tensor_copy(alpha_b[:], bc_ps[:, 0:1])
            nc.vector.tensor_copy(oma_b[:], bc_ps[:, 1:2])
            nc.vector.tensor_scalar(scb[:], edz[:], oma_b[:], None, ALU.mult)

            # ---------------- pass 2 (DMA-prefetched, skew 1) --------------
            steps = ntb * nch
            front = {}

            def p2_front(k):
                tb, c = divmod(k, nch)
                rs, ds_ = tb * P, c * w
                es = i16p.tile([P, w], I16, tag="es")
                nc.sync.dma_start(
                    out=es[:], in_=es_dram[rs:rs + P, ds_:ds_ + w]
                )
                front[k] = es

            def p2_back(k):
                tb, c = divmod(k, nch)
                es = front.pop(k)
                # f = Ln(sc_t * e0 + a) with sc_t = (1-a)*e^{dz_t}
                ft = scr.tile([P, w], BF16, tag="ft")
                nc.scalar.activation(
                    out=ft[:], in_=es[:].bitcast(BF16), func=ACTF.Ln,
                    bias=alpha_b[:], scale=scb[:, tb:tb + 1],
                )
                # (P~ * sct) * f, free-dim-summed into tac; the dummy
                # output overwrites the dead es tile (accum is f32-internal)
                nc.vector.scalar_tensor_tensor(
                    out=es[:],
                    in0=ptil[:, (tb * v + c * w): (tb * v + c * w + w)],
                    scalar=sct[:, tb:tb + 1], in1=ft[:],
                    op0=ALU.mult, op1=ALU.mult,
                    accum_out=tac[:, c:c + 1],
                )
                if c == nch - 1:
                    nc.vector.reduce_sum(
                        out=term[:, tb:tb + 1], in_=tac[:], axis=AX.X
                    )

            for k in range(steps + 1):
                if k < steps:
                    p2_front(k)
                if k >= 1:
                    p2_back(k - 1)

            # core partial = sum_t mask * term
            tmr = blk.tile([P, ntb], F32, tag="tmr")
            tmc = blk.tile([P, 1], F32, tag="tmc")
            nc.vector.tensor_mul(tmr[:], term[:], mask[:])
            nc.vector.reduce_sum(out=tmc[:], in_=tmr[:], axis=AX.X)
            ps1 = psp.tile([1, 1], F32, tag="ps1")
            nc.tensor.matmul(ps1[:], ones[:], tmc[:])
            nc.vector.tensor_copy(out_sb[0:1, 0:1], ps1[:])
            nc.vector.tensor_copy(out_sb[0:1, 1:3], allr[:])
            nc.vector.tensor_copy(out_sb[0:1, 3:4], al[:])
            nc.sync.dma_start(out=out_d[:], in_=out_sb[:])

    nc.compile()
    return nc


def host_prepare(student, teacher, labels, in_dt="e3"):
    """Per-core input maps. Sharding + fp8/bf16 transport cast on host."""
    student = np.asarray(student, dtype=np.float32)
    teacher = np.asarray(teacher, dtype=np.float32)
    labels = np.asarray(labels)
    ntb = TPC // P
    np_dt = {"e3": ml_dtypes.float8_e3m4, "bf16": ml_dtypes.bfloat16,
             "f32": np.float32}[in_dt]
    in_maps = []
    invms = []
    for core in range(NCORES):
        r, h = core // 2, core % 2
        if r % 2 == 0:
            x_full, y_full = teacher[r], student[r]
        else:
            x_full, y_full = student[r], teacher[r]
        sl = slice(h * TPC, (h + 1) * TPC)
        x = np.ascontiguousarray(x_full[sl]).astype(np_dt)
        y = np.ascontiguousarray(y_full[sl]).astype(np_dt)
        t_global = h * TPC + np.arange(TPC)
        valid = t_global <= T - 2
        lbl = np.where(valid, labels[r][np.minimum(t_global + 1, T - 1)], 0)
        m = ((lbl != IGNORE) & valid).astype(np.float32)
        lbl_c = np.clip(lbl, 0, V - 1)
        # gather from the device-visible (cast) values; -1 matches zx = Zx-1
        xlab = x[np.arange(TPC), lbl_c].astype(np.float32) - 1.0
        ylab = y[np.arange(TPC), lbl_c].astype(np.float32) - 1.0
        row_lbl = labels[r][1:]
        mask_total = float(np.maximum((row_lbl != IGNORE).sum(), 1.0))
        invms.append(1.0 / mask_total)

        def fold(vec):
            return np.ascontiguousarray(vec.reshape(ntb, P).T.astype(np.float32))

        in_maps.append({
            "x": x,
            "y": y,
            "xlab": fold(xlab),
            "ylab": fold(ylab),
            "mask": fold(m),
            "invm": np.array([[1.0 / mask_total]], dtype=np.float32),
        })
    return in_maps, invms


def host_combine(results, invms):
    partials = [float(results[i]["out"][0, 0]) for i in range(NCORES)]
    row_vals = []
    for r in range(B):
        pA, pB = partials[2 * r], partials[2 * r + 1]
        row_vals.append(-(pA + pB) * invms[2 * r])
    loss = (2.0 - BETA) * (row_vals[0] + row_vals[2]) / 2.0 \
        + BETA * (row_vals[1] + row_vals[3]) / 2.0
    return np.array(loss, dtype=np.float32)


_NC = None
LAST_RESULT = None  # BassKernelResults from the most recent run (for profiling)


def kernel(student_logits=None, teacher_logits=None, labels=None):
    global _NC, LAST_RESULT
    in_dt = os.environ.get("KERNEL_IN_DT", "e3")
    if _NC is None:
        _NC = build_nc(
            in_dt=in_dt,
            warmup_cc=os.environ.get("KERNEL_WARMUP_CC", "1") == "1",
        )
    in_maps, invms = host_prepare(
        student_logits, teacher_logits, labels, in_dt=in_dt
    )
    res = run_bass_kernel_spmd(
        _NC, in_maps, core_ids=list(range(NCORES)),
        trace=bool(os.environ.get("KERNEL_TRACE")),
    )
    LAST_RESULT = res
    return host_combine(res.results, invms)


# revision 8
# speedup vs baseline: 1.3153x; 1.1380x over previous
"""Distilled-KL loss head on 8 TRN2 NeuronCores — v5.

Math (validated vs the jax reference, see numsim.py):
  For batch row r, with x = teacher logits (even r) / student (odd r), y the
  other tensor, the per-row term is
      rowval = -(1/mask_tot) * sum_t mask_t * sct_t * sum_v P~_v * f_v
  where P~ = e4m3(exp(x - 1))   [fp8 prob cache, written free by pass-1 exp]
        sct = 1 / sum_v fp32_accum(exp(x-1)) = e^{1-Zx}
        f = Ln((1-a)*e + a),  e ~= exp(D~ + dz),  D~ = bf16(y - x),
        dz = Zx - Zy,  a = clip(1 - 0.9/(exp((Sx-Sy)/mask_tot)+1e-5), .01, .1)
  loss = (2-BETA)*mean(rowval even) + BETA*mean(rowval odd).

Performance structure (853us baseline -> v2 571us -> v4):
  * ACT transcendentals 5/elem -> 3/elem:
      - pass-2 p=exp(x-Zx) comes from the persistent fp8 P~ tile
        (125 KB/partition SBUF) written as the pass-1 sumexp's activation
        output (bias=-1 keeps e^(x-1) <= 134 < 240 e4m3 max);
      - pass-2 e=exp(D~+dz) is a Schraudolph fast-exp on the idle DVE:
        one 4x tensor_scalar (v = D~*(2^7/ln2) + (dz*S+C)) -> int16
        round-to-nearest whose bit pattern IS bf16(e); log-mean-centered C,
        +-4% sawtooth cancels in the p-weighted sum (9.5e-4 end-to-end).
  * Inputs pre-cast to fp8 e3m4 on host (transport/sharding choice):
    pass-1 HBM read is 32.8 MB/core; with the bf16 D~ bounce write
    (32.8 MB round trip) pass 1 stays under the 358 GB/s HBM-per-core cap.
  * The sawtooth runs in PASS 1 right after the sub (dz folded instead
    into the Ln's per-token scale: ft = Ln(sc_t*e0 + a), sc_t =
    (1-a)*e^{dz_t}), so the DRAM bounce carries finished sawtooth bits and
    pass-2 DVE is stt-only (no ACT/DVE ping-pong).
  * Pass-1 input DMAs are 2 MB (8000-wide fp8 tiles, two 4000 compute
    slices) to stay on the fast side of the DMA-efficiency knee.
  * mul+reduce fused into one scalar_tensor_tensor with f32 accum_out,
    folding the per-token sct scalar; its dummy output overwrites the dead
    es tile (zero extra SBUF, no WAR hazards).
  * Dummy warmup AllReduce at kernel start pre-pays collective ring setup,
    shrinking the pass-boundary alpha bubble.

Sharding: core c handles batch row c//2, token half c%2 (512 of the 1023
valid shifted tokens; slot 1023 masked). The x/y role swap makes the SPMD
graph identical on all 8 cores. Only cross-core exchange: [1,2] AllReduce of
(Sx, Sy) within each core pair.
"""
import os
import numpy as np
import ml_dtypes

import bass_rust as _bass_rust
from concourse import bacc, tile, mybir
from concourse.bass_utils import run_bass_kernel_spmd
from concourse.hw_specs import get_activation_tables


class _OneActSetBacc(bacc.Bacc):
    """Force Exp and Ln to resolve to the single act-function set that
    contains both (``natural_log_exp_and_others``), so alternating Exp/Ln
    activations emit zero ACT_TABLE_LOADs after the first. Entry order is
    preserved so act_func_set_id indices still match act_info.json."""

    def insert_act_table_loads(self):
        has_activation = any(
            isinstance(i, mybir.InstActivation)
            for b in self.main_func.blocks
            for i in b.instructions
        )
        if not has_activation:
            return
        tables = get_activation_tables(self.m.arch)
        both = "natural_log_exp_and_others"
        exp, ln = (
            mybir.ActivationFunctionType.Exp,
            mybir.ActivationFunctionType.Ln,
        )
        if both in tables and {exp, ln} <= tables[both]:
            tables = {
                name: (fns if name == both else fns - {exp, ln})
                for name, fns in tables.items()
            }
        _bass_rust.insert_act_table_loads(self, list(tables.items()))


B, T, V = 4, 1024, 32000
P = 128                 # SBUF partitions = tokens per block
TPC = 512               # token slots per core
NCORES = 8
IGNORE = -100
BASE_ALPHA = 0.1
BETA = 1.0
F32 = mybir.dt.float32
BF16 = mybir.dt.bfloat16
E4 = mybir.dt.float8e4
E3 = mybir.dt.float8e3
I16 = mybir.dt.int16
AX = mybir.AxisListType
ALU = mybir.AluOpType
ACTF = mybir.ActivationFunctionType

REPLICA_GROUPS = [[0, 1], [2, 3], [4, 5], [6, 7]]
SAW_S = 184.6649652337873     # 2^7/ln2
SAW_C = 16256.0 - 7.335       # 127*2^7, log-mean-centered


def build_nc(tpc=TPC, v=V, in_dt="e3", warmup_cc=True):
    IN_DT = {"e3": E3, "bf16": BF16, "f32": F32}[in_dt]
    TILE_DT = BF16 if in_dt == "f32" else IN_DT  # f32 mode: SWDGE cast to bf16
    w = 4000 if in_dt == "e3" else 2000
    w_dma = 2 * w if in_dt == "e3" else w   # 2 MB input DMAs in fp8 mode
    spc = w_dma // w
    ntb = tpc // P
    nch = v // w
    ndc = v // w_dma
    assert ntb * P == tpc and nch * w == v and ndc * w_dma == v

    nc = _OneActSetBacc(
        "TRN2", target_bir_lowering=False, debug=False, num_devices=NCORES
    )
    x_d = nc.dram_tensor("x", [tpc, v], IN_DT, kind="ExternalInput")
    y_d = nc.dram_tensor("y", [tpc, v], IN_DT, kind="ExternalInput")
    xlab_d = nc.dram_tensor("xlab", [P, ntb], F32, kind="ExternalInput")
    ylab_d = nc.dram_tensor("ylab", [P, ntb], F32, kind="ExternalInput")
    mask_d = nc.dram_tensor("mask", [P, ntb], F32, kind="ExternalInput")
    invm_d = nc.dram_tensor("invm", [1, 1], F32, kind="ExternalInput")
    out_d = nc.dram_tensor("out", [1, 4], F32, kind="ExternalOutput")

    with tile.TileContext(nc) as tc:
        with (
            tc.tile_pool(name="big", bufs=2) as big,
            tc.tile_pool(name="dst", bufs=1) as dstp,
            tc.tile_pool(name="ef", bufs=2) as scr,
            tc.tile_pool(name="i16", bufs=3) as i16p,
            tc.tile_pool(name="blk", bufs=2) as blk,
            tc.tile_pool(name="sm", bufs=1) as sm,
            tc.tile_pool(name="psum", bufs=2, space="PSUM") as psp,
            tc.tile_pool(name="dram", bufs=2, space="DRAM") as dram,
        ):
            # persistent tiles
            ptil = sm.tile([P, ntb * v], E4, tag="ptil")  # e4m3 cache of e^(x-1)
            zx = sm.tile([P, ntb], F32, tag="zx")         # ln sumexp(x-1) = Zx-1
            zy = sm.tile([P, ntb], F32, tag="zy")
            dz = sm.tile([P, ntb], F32, tag="dz")         # Zx - Zy
            edz = sm.tile([P, ntb], F32, tag="edz")       # e^{dz}
            scb = sm.tile([P, ntb], F32, tag="scb")       # (1-a)*e^{dz}
            sct = sm.tile([P, ntb], F32, tag="sct")       # e^{1-Zx}
            axc = sm.tile([P, nch], F32, tag="axc")       # per-chunk sumexp(x-1)
            ayc = sm.tile([P, nch], F32, tag="ayc")
            tac = sm.tile([P, nch], F32, tag="tac")       # per-chunk sct*sum P~ f
            term = sm.tile([P, ntb], F32, tag="term")     # per-token sum_v p*f
            xlab = sm.tile([P, ntb], F32, tag="xlab")     # host: x[t,lbl]-1
            ylab = sm.tile([P, ntb], F32, tag="ylab")
            mask = sm.tile([P, ntb], F32, tag="mask")
            sxsy = sm.tile([P, 2], F32, tag="sxsy")
            ones = sm.tile([P, 1], F32, tag="ones")
            ones_row = sm.tile([1, P], F32, tag="ones_row")
            neg1 = sm.tile([P, 1], F32, tag="neg1")
            invm_sb = sm.tile([1, 1], F32, tag="invm_sb")
            allr = sm.tile([1, 2], F32, tag="allr")       # allreduced (Sx, Sy)
            alpha_b = sm.tile([P, 1], F32, tag="alpha_b")
            oma_b = sm.tile([P, 1], F32, tag="oma_b")
            out_sb = sm.tile([1, 4], F32, tag="out_sb")

            nc.vector.memset(ones[:], 1.0)
            nc.vector.memset(ones_row[:], 1.0)
            nc.vector.memset(neg1[:], -1.0)
            nc.sync.dma_start(out=xlab[:], in_=xlab_d[:])
            nc.sync.dma_start(out=ylab[:], in_=ylab_d[:])
            nc.sync.dma_start(out=mask[:], in_=mask_d[:])
            nc.sync.dma_start(out=invm_sb[:], in_=invm_d[:])

            if warmup_cc:
                # dummy AllReduce to pre-pay collective ring setup; runs
                # concurrently with early pass-1 compute
                wsb = sm.tile([1, 1], F32, tag="wsb")
                wjk = sm.tile([1, 1], F32, tag="wjk")
                nc.vector.memset(wsb[:], 0.0)
                w_in = dram.tile([1, 1], F32, tag="w_in")
                w_out = dram.tile([1, 1], F32, tag="w_out")
                nc.sync.dma_start(out=w_in[:], in_=wsb[:])
                nc.gpsimd.collective_compute(
                    "AllReduce", ALU.add, replica_groups=REPLICA_GROUPS,
                    ins=[w_in[:].opt()], outs=[w_out[:].opt()],
                )
                nc.sync.dma_start(out=wjk[:], in_=w_out[:])

            es_dram = dram.tile([tpc, v], I16, tag="es_dram")  # sawtooth-bits bounce

            in_dma = nc.gpsimd if in_dt == "f32" else nc.sync

            # ---------------- pass 1 ----------------
            for tb in range(ntb):
                rs = tb * P
                for cd in range(ndc):
                    dd = cd * w_dma
                    xt = big.tile([P, w_dma], TILE_DT, tag="xt")
                    in_dma.dma_start(out=xt[:], in_=x_d[rs:rs + P, dd:dd + w_dma])
                    yt = big.tile([P, w_dma], TILE_DT, tag="yt")
                    in_dma.dma_start(out=yt[:], in_=y_d[rs:rs + P, dd:dd + w_dma])
                    for sp in range(spc):
                        c = cd * spc + sp
                        ds_ = c * w
                        sl = slice(sp * w, (sp + 1) * w)
                        # sumexp(x-1) with the e4m3 prob cache as free output
                        nc.scalar.activation(
                            out=ptil[:, tb * v + ds_: tb * v + ds_ + w],
                            in_=xt[:, sl], func=ACTF.Exp, bias=neg1[:],
                            accum_out=axc[:, c:c + 1],
                        )
                        es = i16p.tile([P, w], I16, tag="es")
                        # exp(y-1) junk output lands in the es tile (bf16
                        # view) -- only the f32 accum matters; ts overwrites
                        nc.scalar.activation(
                            out=es[:].bitcast(BF16), in_=yt[:, sl],
                            func=ACTF.Exp, bias=neg1[:],
                            accum_out=ayc[:, c:c + 1],
                        )
                        dt = dstp.tile([P, w], BF16, tag="dt")
                        nc.vector.tensor_sub(dt[:], yt[:, sl], xt[:, sl])
                        # sawtooth bits (dz applied later via the Ln scale)
                        nc.vector.tensor_scalar(
                            es[:], dt[:], SAW_S, SAW_C, ALU.mult, ALU.add,
                        )
                        nc.sync.dma_start(
                            out=es_dram[rs:rs + P, ds_:ds_ + w], in_=es[:]
                        )
                sume_x = blk.tile([P, 1], F32, tag="sume_x")
                nc.vector.reduce_sum(out=sume_x[:], in_=axc[:], axis=AX.X)
                nc.scalar.activation(out=zx[:, tb:tb + 1], in_=sume_x[:], func=ACTF.Ln)
                nc.vector.reciprocal(sct[:, tb:tb + 1], sume_x[:])
                sume_y = blk.tile([P, 1], F32, tag="sume_y")
                nc.vector.reduce_sum(out=sume_y[:], in_=ayc[:], axis=AX.X)
                nc.scalar.activation(out=zy[:, tb:tb + 1], in_=sume_y[:], func=ACTF.Ln)

            nc.vector.tensor_sub(dz[:], zx[:], zy[:])
            nc.scalar.activation(out=edz[:], in_=dz[:], func=ACTF.Exp)

            # label partial sums Sx, Sy over this core's tokens
            # (xlab/ylab arrive host-adjusted by -1 to match zx = Zx-1)
            ptx = blk.tile([P, ntb], F32, tag="ptx")
            nc.vector.tensor_sub(ptx[:], xlab[:], zx[:])
            ttx = blk.tile([P, ntb], F32, tag="ttx")
            nc.vector.tensor_mul(ttx[:], ptx[:], mask[:])
            nc.vector.reduce_sum(out=sxsy[:, 0:1], in_=ttx[:], axis=AX.X)
            pty = blk.tile([P, ntb], F32, tag="pty")
            nc.vector.tensor_sub(pty[:], ylab[:], zy[:])
            tty = blk.tile([P, ntb], F32, tag="tty")
            nc.vector.tensor_mul(tty[:], pty[:], mask[:])
            nc.vector.reduce_sum(out=sxsy[:, 1:2], in_=tty[:], axis=AX.X)
            # partition-reduce via matmul with ones: [128,2] -> [1,2]
            ps2 = psp.tile([1, 2], F32, tag="ps2")
            nc.tensor.matmul(ps2[:], ones[:], sxsy[:])
            sb2 = blk.tile([1, 2], F32, tag="sb2")
            nc.vector.tensor_copy(sb2[:], ps2[:])

            in_bounce = dram.tile([1, 2], F32, tag="in_bounce")
            out_bounce = dram.tile([1, 2], F32, tag="out_bounce")
            nc.sync.dma_start(out=in_bounce[:], in_=sb2[:])
            nc.gpsimd.collective_compute(
                "AllReduce", ALU.add, replica_groups=REPLICA_GROUPS,
                ins=[in_bounce[:].opt()], outs=[out_bounce[:].opt()],
            )
            nc.sync.dma_start(out=allr[:], in_=out_bounce[:])

            # alpha = clip(1 - 0.9/(exp((Sx-Sy)*invm) + 1e-5), 0.01, 0.1)
            t1 = blk.tile([1, 1], F32, tag="t1")
            nc.vector.tensor_sub(t1[:], allr[0:1, 0:1], allr[0:1, 1:2])
            t2 = blk.tile([1, 1], F32, tag="t2")
            nc.vector.tensor_mul(t2[:], t1[:], invm_sb[:])
            t3 = blk.tile([1, 1], F32, tag="t3")
            nc.scalar.activation(out=t3[:], in_=t2[:], func=ACTF.Exp)
            t4 = blk.tile([1, 1], F32, tag="t4")
            nc.vector.tensor_scalar_add(t4[:], t3[:], 1e-5)
            t5 = blk.tile([1, 1], F32, tag="t5")
            nc.vector.reciprocal(t5[:], t4[:])
            t6 = blk.tile([1, 1], F32, tag="t6")
            nc.vector.tensor_scalar(
                t6[:], t5[:], -(1.0 - BASE_ALPHA), 1.0, ALU.mult, ALU.add
            )
            al = blk.tile([1, 1], F32, tag="al")
            nc.vector.tensor_scalar(
                al[:], t6[:], BASE_ALPHA, 0.01, ALU.min, ALU.max
            )
            om = blk.tile([1, 1], F32, tag="om")
            nc.vector.tensor_scalar(om[:], al[:], -1.0, 1.0, ALU.mult, ALU.add)
            alom = blk.tile([1, 2], F32, tag="alom")
            nc.vector.tensor_copy(alom[0:1, 0:1], al[:])
            nc.vector.tensor_copy(alom[0:1, 1:2], om[:])
            bc_ps = psp.tile([P, 2], F32, tag="bc_ps")
            nc.tensor.matmul(bc_ps[:], ones_row[:], alom[:])
            nc.vector.tensor_copy(alpha_b[:], bc_ps[:, 0:1])
            nc.vector.tensor_copy(oma_b[:], bc_ps[:, 1:2])
            nc.vector.tensor_scalar(scb[:], edz[:], oma_b[:], None, ALU.mult)

            # ---------------- pass 2 (DMA-prefetched, skew 1) --------------
            steps = ntb * nch
            front = {}

            def p2_front(k):
                tb, c = divmod(k, nch)
                rs, ds_ = tb * P, c * w
                es = i16p.tile([P, w], I16, tag="es")
                nc.sync.dma_start(
                    out=es[:], in_=es_dram[rs:rs + P, ds_:ds_ + w]
                )
                front[k] = es

            def p2_back(k):
                tb, c = divmod(k, nch)
                es = front.pop(k)
                # f = Ln(sc_t * e0 + a) with sc_t = (1-a)*e^{dz_t}
                ft = scr.tile([P, w], BF16, tag="ft")
                nc.scalar.activation(
                    out=ft[:], in_=es[:].bitcast(BF16), func=ACTF.Ln,
                    bias=alpha_b[:], scale=scb[:, tb:tb + 1],
                )
                # (P~ * sct) * f, free-dim-summed into tac; the dummy
                # output overwrites the dead es tile (accum is f32-internal)
                nc.vector.scalar_tensor_tensor(
                    out=es[:],
                    in0=ptil[:, (tb * v + c * w): (tb * v + c * w + w)],
                    scalar=sct[:, tb:tb + 1], in1=ft[:],
                    op0=ALU.mult, op1=ALU.mult,
                    accum_out=tac[:, c:c + 1],
                )
                if c == nch - 1:
                    nc.vector.reduce_sum(
                        out=term[:, tb:tb + 1], in_=tac[:], axis=AX.X
                    )

            for k in range(steps + 1):
                if k < steps:
                    p2_front(k)
                if k >= 1:
                    p2_back(k - 1)

            # core partial = sum_t mask * term
            tmr = blk.tile([P, ntb], F32, tag="tmr")
            tmc = blk.tile([P, 1], F32, tag="tmc")
            nc.vector.tensor_mul(tmr[:], term[:], mask[:])
            nc.vector.reduce_sum(out=tmc[:], in_=tmr[:], axis=AX.X)
            ps1 = psp.tile([1, 1], F32, tag="ps1")
            nc.tensor.matmul(ps1[:], ones[:], tmc[:])
            nc.vector.tensor_copy(out_sb[0:1, 0:1], ps1[:])
            nc.vector.tensor_copy(out_sb[0:1, 1:3], allr[:])
            nc.vector.tensor_copy(out_sb[0:1, 3:4], al[:])
            nc.sync.dma_start(out=out_d[:], in_=out_sb[:])

    nc.compile()
    return nc


def host_prepare(student, teacher, labels, in_dt="e3"):
    """Per-core input maps. Sharding + fp8/bf16 transport cast on host."""
    student = np.asarray(student, dtype=np.float32)
    teacher = np.asarray(teacher, dtype=np.float32)
    labels = np.asarray(labels)
    ntb = TPC // P
    np_dt = {"e3": ml_dtypes.float8_e3m4, "bf16": ml_dtypes.bfloat16,
             "f32": np.float32}[in_dt]
    in_maps = []
    invms = []
    for core in range(NCORES):
        r, h = core // 2, core % 2
        if r % 2 == 0:
            x_full, y_full = teacher[r], student[r]
        else:
            x_full, y_full = student[r], teacher[r]
        sl = slice(h * TPC, (h + 1) * TPC)
        x = np.ascontiguousarray(x_full[sl]).astype(np_dt)
        y = np.ascontiguousarray(y_full[sl]).astype(np_dt)
        t_global = h * TPC + np.arange(TPC)
        valid = t_global <= T - 2
        lbl = np.where(valid, labels[r][np.minimum(t_global + 1, T - 1)], 0)
        m = ((lbl != IGNORE) & valid).astype(np.float32)
        lbl_c = np.clip(lbl, 0, V - 1)
        # gather from the device-visible (cast) values; -1 matches zx = Zx-1
        xlab = x[np.arange(TPC), lbl_c].astype(np.float32) - 1.0
        ylab = y[np.arange(TPC), lbl_c].astype(np.float32) - 1.0
        row_lbl = labels[r][1:]
        mask_total = float(np.maximum((row_lbl != IGNORE).sum(), 1.0))
        invms.append(1.0 / mask_total)

        def fold(vec):
            return np.ascontiguousarray(vec.reshape(ntb, P).T.astype(np.float32))

        in_maps.append({
            "x": x,
            "y": y,
            "xlab": fold(xlab),
            "ylab": fold(ylab),
            "mask": fold(m),
            "invm": np.array([[1.0 / mask_total]], dtype=np.float32),
        })
    return in_maps, invms


def host_combine(results, invms):
    partials = [float(results[i]["out"][0, 0]) for i in range(NCORES)]
    row_vals = []
    for r in range(B):
        pA, pB = partials[2 * r], partials[2 * r + 1]
        row_vals.append(-(pA + pB) * invms[2 * r])
    loss = (2.0 - BETA) * (row_vals[0] + row_vals[2]) / 2.0 \
        + BETA * (row_vals[1] + row_vals[3]) / 2.0
    return np.array(loss, dtype=np.float32)


_NC = None
LAST_RESULT = None  # BassKernelResults from the most recent run (for profiling)


def kernel(student_logits=None, teacher_logits=None, labels=None):
    global _NC, LAST_RESULT
    in_dt = os.environ.get("KERNEL_IN_DT", "e3")
    if _NC is None:
        _NC = build_nc(
            in_dt=in_dt,
            warmup_cc=os.environ.get("KERNEL_WARMUP_CC", "1") == "1",
        )
    in_maps, invms = host_prepare(
        student_logits, teacher_logits, labels, in_dt=in_dt
    )
    res = run_bass_kernel_spmd(
        _NC, in_maps, core_ids=list(range(NCORES)),
        trace=bool(os.environ.get("KERNEL_TRACE")),
    )
    LAST_RESULT = res
    return host_combine(res.results, invms)


# revision 9
# speedup vs baseline: 1.3345x; 1.0146x over previous
"""Distilled-KL loss head on 8 TRN2 NeuronCores — v5.

Math (validated vs the jax reference, see numsim.py):
  For batch row r, with x = teacher logits (even r) / student (odd r), y the
  other tensor, the per-row term is
      rowval = -(1/mask_tot) * sum_t mask_t * sct_t * sum_v P~_v * f_v
  where P~ = e4m3(exp(x - 1))   [fp8 prob cache, written free by pass-1 exp]
        sct = 1 / sum_v fp32_accum(exp(x-1)) = e^{1-Zx}
        f = Ln((1-a)*e + a),  e ~= exp(D~ + dz),  D~ = bf16(y - x),
        dz = Zx - Zy,  a = clip(1 - 0.9/(exp((Sx-Sy)/mask_tot)+1e-5), .01, .1)
  loss = (2-BETA)*mean(rowval even) + BETA*mean(rowval odd).

Performance structure (853us baseline -> v2 571us -> v4):
  * ACT transcendentals 5/elem -> 3/elem:
      - pass-2 p=exp(x-Zx) comes from the persistent fp8 P~ tile
        (125 KB/partition SBUF) written as the pass-1 sumexp's activation
        output (bias=-1 keeps e^(x-1) <= 134 < 240 e4m3 max);
      - pass-2 e=exp(D~+dz) is a Schraudolph fast-exp on the idle DVE:
        one 4x tensor_scalar (v = D~*(2^7/ln2) + (dz*S+C)) -> int16
        round-to-nearest whose bit pattern IS bf16(e); log-mean-centered C,
        +-4% sawtooth cancels in the p-weighted sum (9.5e-4 end-to-end).
  * Inputs pre-cast to fp8 e3m4 on host (transport/sharding choice):
    pass-1 HBM read is 32.8 MB/core; with the bf16 D~ bounce write
    (32.8 MB round trip) pass 1 stays under the 358 GB/s HBM-per-core cap.
  * The sawtooth runs in PASS 1 right after the sub (dz folded instead
    into the Ln's per-token scale: ft = Ln(sc_t*e0 + a), sc_t =
    (1-a)*e^{dz_t}), so the DRAM bounce carries finished sawtooth bits and
    pass-2 DVE is stt-only (no ACT/DVE ping-pong).
  * Pass-1 input DMAs are 2 MB (8000-wide fp8 tiles, two 4000 compute
    slices) to stay on the fast side of the DMA-efficiency knee.
  * mul+reduce fused into one scalar_tensor_tensor with f32 accum_out,
    folding the per-token sct scalar; its dummy output overwrites the dead
    es tile (zero extra SBUF, no WAR hazards).
  * Dummy warmup AllReduce at kernel start pre-pays collective ring setup,
    shrinking the pass-boundary alpha bubble.

Sharding: core c handles batch row c//2, token half c%2 (512 of the 1023
valid shifted tokens; slot 1023 masked). The x/y role swap makes the SPMD
graph identical on all 8 cores. Only cross-core exchange: [1,2] AllReduce of
(Sx, Sy) within each core pair.
"""
import os
import numpy as np
import ml_dtypes

import bass_rust as _bass_rust
from concourse import bacc, tile, mybir
from concourse.bass_utils import run_bass_kernel_spmd
from concourse.hw_specs import get_activation_tables


class _OneActSetBacc(bacc.Bacc):
    """Force Exp and Ln to resolve to the single act-function set that
    contains both (``natural_log_exp_and_others``), so alternating Exp/Ln
    activations emit zero ACT_TABLE_LOADs after the first. Entry order is
    preserved so act_func_set_id indices still match act_info.json."""

    def insert_act_table_loads(self):
        has_activation = any(
            isinstance(i, mybir.InstActivation)
            for b in self.main_func.blocks
            for i in b.instructions
        )
        if not has_activation:
            return
        tables = get_activation_tables(self.m.arch)
        both = "natural_log_exp_and_others"
        exp, ln = (
            mybir.ActivationFunctionType.Exp,
            mybir.ActivationFunctionType.Ln,
        )
        if both in tables and {exp, ln} <= tables[both]:
            tables = {
                name: (fns if name == both else fns - {exp, ln})
                for name, fns in tables.items()
            }
        _bass_rust.insert_act_table_loads(self, list(tables.items()))


B, T, V = 4, 1024, 32000
P = 128                 # SBUF partitions = tokens per block
TPC = 512               # token slots per core
NCORES = 8
IGNORE = -100
BASE_ALPHA = 0.1
BETA = 1.0
F32 = mybir.dt.float32
BF16 = mybir.dt.bfloat16
E4 = mybir.dt.float8e4
E3 = mybir.dt.float8e3
I16 = mybir.dt.int16
AX = mybir.AxisListType
ALU = mybir.AluOpType
ACTF = mybir.ActivationFunctionType

REPLICA_GROUPS = [[0, 1], [2, 3], [4, 5], [6, 7]]
SAW_S = 184.6649652337873     # 2^7/ln2
SAW_C = 16256.0 - 7.335       # 127*2^7, log-mean-centered


def build_nc(tpc=TPC, v=V, in_dt="e3", warmup_cc=True):
    IN_DT = {"e3": E3, "bf16": BF16, "f32": F32}[in_dt]
    TILE_DT = BF16 if in_dt == "f32" else IN_DT  # f32 mode: SWDGE cast to bf16
    w = 4000 if in_dt == "e3" else 2000
    w_dma = 2 * w if in_dt == "e3" else w   # 2 MB input DMAs in fp8 mode
    spc = w_dma // w
    ntb = tpc // P
    nch = v // w
    ndc = v // w_dma
    assert ntb * P == tpc and nch * w == v and ndc * w_dma == v

    nc = _OneActSetBacc(
        "TRN2", target_bir_lowering=False, debug=False, num_devices=NCORES
    )
    x_d = nc.dram_tensor("x", [tpc, v], IN_DT, kind="ExternalInput")
    y_d = nc.dram_tensor("y", [tpc, v], IN_DT, kind="ExternalInput")
    xlab_d = nc.dram_tensor("xlab", [P, ntb], F32, kind="ExternalInput")
    ylab_d = nc.dram_tensor("ylab", [P, ntb], F32, kind="ExternalInput")
    mask_d = nc.dram_tensor("mask", [P, ntb], F32, kind="ExternalInput")
    invm_d = nc.dram_tensor("invm", [1, 1], F32, kind="ExternalInput")
    out_d = nc.dram_tensor("out", [1, 4], F32, kind="ExternalOutput")

    with tile.TileContext(nc) as tc:
        with (
            tc.tile_pool(name="big", bufs=2) as big,
            tc.tile_pool(name="dst", bufs=1) as dstp,
            tc.tile_pool(name="ef", bufs=2) as scr,
            tc.tile_pool(name="i16", bufs=3) as i16p,
            tc.tile_pool(name="blk", bufs=2) as blk,
            tc.tile_pool(name="sm", bufs=1) as sm,
            tc.tile_pool(name="psum", bufs=2, space="PSUM") as psp,
            tc.tile_pool(name="dram", bufs=2, space="DRAM") as dram,
        ):
            # persistent tiles
            ptil = sm.tile([P, ntb * v], E4, tag="ptil")  # e4m3 cache of e^(x-1)
            zx = sm.tile([P, ntb], F32, tag="zx")         # ln sumexp(x-1) = Zx-1
            zy = sm.tile([P, ntb], F32, tag="zy")
            dz = sm.tile([P, ntb], F32, tag="dz")         # Zx - Zy
            edz = sm.tile([P, ntb], F32, tag="edz")       # e^{dz}
            scb = sm.tile([P, ntb], F32, tag="scb")       # (1-a)*e^{dz}
            sct = sm.tile([P, ntb], F32, tag="sct")       # e^{1-Zx}
            axc = sm.tile([P, nch], F32, tag="axc")       # per-dma-chunk sumexp(x-1)
            ayc = sm.tile([P, nch], F32, tag="ayc")
            tac = sm.tile([P, nch], F32, tag="tac")       # per-chunk sct*sum P~ f
            term = sm.tile([P, ntb], F32, tag="term")     # per-token sum_v p*f
            xlab = sm.tile([P, ntb], F32, tag="xlab")     # host: x[t,lbl]-1
            ylab = sm.tile([P, ntb], F32, tag="ylab")
            mask = sm.tile([P, ntb], F32, tag="mask")
            sxsy = sm.tile([P, 2], F32, tag="sxsy")
            ones = sm.tile([P, 1], F32, tag="ones")
            ones_row = sm.tile([1, P], F32, tag="ones_row")
            neg1 = sm.tile([P, 1], F32, tag="neg1")
            invm_sb = sm.tile([1, 1], F32, tag="invm_sb")
            allr = sm.tile([1, 2], F32, tag="allr")       # allreduced (Sx, Sy)
            alpha_b = sm.tile([P, 1], F32, tag="alpha_b")
            oma_b = sm.tile([P, 1], F32, tag="oma_b")
            out_sb = sm.tile([1, 4], F32, tag="out_sb")

            nc.vector.memset(ones[:], 1.0)
            nc.vector.memset(ones_row[:], 1.0)
            nc.vector.memset(neg1[:], -1.0)
            nc.sync.dma_start(out=xlab[:], in_=xlab_d[:])
            nc.sync.dma_start(out=ylab[:], in_=ylab_d[:])
            nc.sync.dma_start(out=mask[:], in_=mask_d[:])
            nc.sync.dma_start(out=invm_sb[:], in_=invm_d[:])

            if warmup_cc:
                # dummy AllReduce to pre-pay collective ring setup; runs
                # concurrently with early pass-1 compute
                wsb = sm.tile([1, 1], F32, tag="wsb")
                wjk = sm.tile([1, 1], F32, tag="wjk")
                nc.vector.memset(wsb[:], 0.0)
                w_in = dram.tile([1, 1], F32, tag="w_in")
                w_out = dram.tile([1, 1], F32, tag="w_out")
                nc.sync.dma_start(out=w_in[:], in_=wsb[:])
                nc.gpsimd.collective_compute(
                    "AllReduce", ALU.add, replica_groups=REPLICA_GROUPS,
                    ins=[w_in[:].opt()], outs=[w_out[:].opt()],
                )
                nc.sync.dma_start(out=wjk[:], in_=w_out[:])

            es_dram = dram.tile([tpc, v], I16, tag="es_dram")  # sawtooth-bits bounce

            in_dma = nc.gpsimd if in_dt == "f32" else nc.sync

            # ---------------- pass 1 ----------------
            for tb in range(ntb):
                rs = tb * P
                # tb0 starts with two smaller DMA chunks so the first exp
                # fires ~6us earlier (pipeline cold-start)
                if tb == 0 and w_dma > w:
                    widths = [w, w] + [w_dma] * (ndc - 1)
                else:
                    widths = [w_dma] * ndc
                off = 0
                nax = 0
                for wd in widths:
                    dd = off
                    xt = big.tile([P, wd], TILE_DT, tag="xt")
                    in_dma.dma_start(out=xt[:], in_=x_d[rs:rs + P, dd:dd + wd])
                    yt = big.tile([P, wd], TILE_DT, tag="yt")
                    in_dma.dma_start(out=yt[:], in_=y_d[rs:rs + P, dd:dd + wd])
                    # wide sumexp(x-1): e4m3 prob cache as free output
                    nc.scalar.activation(
                        out=ptil[:, tb * v + dd: tb * v + dd + wd],
                        in_=xt[:], func=ACTF.Exp, bias=neg1[:],
                        accum_out=axc[:, nax:nax + 1],
                    )
                    nax += 1
                    for sp in range(wd // w):
                        c = (dd + sp * w) // w
                        ds_ = c * w
                        sl = slice(sp * w, (sp + 1) * w)
                        es = i16p.tile([P, w], I16, tag="es")
                        # exp(y-1) junk output lands in the es tile (bf16
                        # view) -- only the f32 accum matters; ts overwrites
                        nc.scalar.activation(
                            out=es[:].bitcast(BF16), in_=yt[:, sl],
                            func=ACTF.Exp, bias=neg1[:],
                            accum_out=ayc[:, c:c + 1],
                        )
                        dt = dstp.tile([P, w], BF16, tag="dt")
                        nc.vector.tensor_sub(dt[:], yt[:, sl], xt[:, sl])
                        # sawtooth bits (dz applied later via the Ln scale)
                        nc.vector.tensor_scalar(
                            es[:], dt[:], SAW_S, SAW_C, ALU.mult, ALU.add,
                        )
                        nc.sync.dma_start(
                            out=es_dram[rs:rs + P, ds_:ds_ + w], in_=es[:]
                        )
                    off += wd
                sume_x = blk.tile([P, 1], F32, tag="sume_x")
                nc.vector.reduce_sum(out=sume_x[:], in_=axc[:, 0:nax], axis=AX.X)
                nc.scalar.activation(out=zx[:, tb:tb + 1], in_=sume_x[:], func=ACTF.Ln)
                nc.vector.reciprocal(sct[:, tb:tb + 1], sume_x[:])
                sume_y = blk.tile([P, 1], F32, tag="sume_y")
                nc.vector.reduce_sum(out=sume_y[:], in_=ayc[:], axis=AX.X)
                nc.scalar.activation(out=zy[:, tb:tb + 1], in_=sume_y[:], func=ACTF.Ln)

            nc.vector.tensor_sub(dz[:], zx[:], zy[:])
            nc.scalar.activation(out=edz[:], in_=dz[:], func=ACTF.Exp)

            # label partial sums Sx, Sy over this core's tokens
            # (xlab/ylab arrive host-adjusted by -1 to match zx = Zx-1)
            ptx = blk.tile([P, ntb], F32, tag="ptx")
            nc.vector.tensor_sub(ptx[:], xlab[:], zx[:])
            ttx = blk.tile([P, ntb], F32, tag="ttx")
            nc.vector.tensor_mul(ttx[:], ptx[:], mask[:])
            nc.vector.reduce_sum(out=sxsy[:, 0:1], in_=ttx[:], axis=AX.X)
            pty = blk.tile([P, ntb], F32, tag="pty")
            nc.vector.tensor_sub(pty[:], ylab[:], zy[:])
            tty = blk.tile([P, ntb], F32, tag="tty")
            nc.vector.tensor_mul(tty[:], pty[:], mask[:])
            nc.vector.reduce_sum(out=sxsy[:, 1:2], in_=tty[:], axis=AX.X)
            # partition-reduce via matmul with ones: [128,2] -> [1,2]
            ps2 = psp.tile([1, 2], F32, tag="ps2")
            nc.tensor.matmul(ps2[:], ones[:], sxsy[:])
            sb2 = blk.tile([1, 2], F32, tag="sb2")
            nc.vector.tensor_copy(sb2[:], ps2[:])

            in_bounce = dram.tile([1, 2], F32, tag="in_bounce")
            out_bounce = dram.tile([1, 2], F32, tag="out_bounce")
            nc.sync.dma_start(out=in_bounce[:], in_=sb2[:])
            nc.gpsimd.collective_compute(
                "AllReduce", ALU.add, replica_groups=REPLICA_GROUPS,
                ins=[in_bounce[:].opt()], outs=[out_bounce[:].opt()],
            )
            nc.sync.dma_start(out=allr[:], in_=out_bounce[:])

            # alpha = clip(1 - 0.9/(exp((Sx-Sy)*invm) + 1e-5), 0.01, 0.1)
            t1 = blk.tile([1, 1], F32, tag="t1")
            nc.vector.tensor_sub(t1[:], allr[0:1, 0:1], allr[0:1, 1:2])
            t3 = blk.tile([1, 1], F32, tag="t3")
            nc.scalar.activation(out=t3[:], in_=t1[:], func=ACTF.Exp,
                                 scale=invm_sb[:])
            t4 = blk.tile([1, 1], F32, tag="t4")
            nc.vector.tensor_scalar_add(t4[:], t3[:], 1e-5)
            t5 = blk.tile([1, 1], F32, tag="t5")
            nc.vector.reciprocal(t5[:], t4[:])
            t6 = blk.tile([1, 1], F32, tag="t6")
            nc.vector.tensor_scalar(
                t6[:], t5[:], -(1.0 - BASE_ALPHA), 1.0, ALU.mult, ALU.add
            )
            alom = blk.tile([1, 2], F32, tag="alom")
            al = alom[0:1, 0:1]
            nc.vector.tensor_scalar(
                al, t6[:], BASE_ALPHA, 0.01, ALU.min, ALU.max
            )
            nc.vector.tensor_scalar(alom[0:1, 1:2], al, -1.0, 1.0,
                                    ALU.mult, ALU.add)
            bc_ps = psp.tile([P, 2], F32, tag="bc_ps")
            nc.tensor.matmul(bc_ps[:], ones_row[:], alom[:])
            nc.vector.tensor_copy(alpha_b[:], bc_ps[:, 0:1])
            nc.vector.tensor_copy(oma_b[:], bc_ps[:, 1:2])
            nc.vector.tensor_scalar(scb[:], edz[:], oma_b[:], None, ALU.mult)

            # ---------------- pass 2 (DMA-prefetched, skew 1) --------------
            steps = ntb * nch
            front = {}

            def p2_front(k):
                tb, c = divmod(k, nch)
                rs, ds_ = tb * P, c * w
                es = i16p.tile([P, w], I16, tag="es")
                nc.sync.dma_start(
                    out=es[:], in_=es_dram[rs:rs + P, ds_:ds_ + w]
                )
                front[k] = es

            def p2_back(k):
                tb, c = divmod(k, nch)
                es = front.pop(k)
                # f = Ln(sc_t * e0 + a) with sc_t = (1-a)*e^{dz_t}
                ft = scr.tile([P, w], BF16, tag="ft")
                nc.scalar.activation(
                    out=ft[:], in_=es[:].bitcast(BF16), func=ACTF.Ln,
                    bias=alpha_b[:], scale=scb[:, tb:tb + 1],
                )
                # (P~ * sct) * f, free-dim-summed into tac; the dummy
                # output overwrites the dead es tile (accum is f32-internal)
                nc.vector.scalar_tensor_tensor(
                    out=es[:],
                    in0=ptil[:, (tb * v + c * w): (tb * v + c * w + w)],
                    scalar=sct[:, tb:tb + 1], in1=ft[:],
                    op0=ALU.mult, op1=ALU.mult,
                    accum_out=tac[:, c:c + 1],
                )
                if c == nch - 1:
                    nc.vector.reduce_sum(
                        out=term[:, tb:tb + 1], in_=tac[:], axis=AX.X
                    )

            for k in range(steps + 1):
                if k < steps:
                    p2_front(k)
                if k >= 1:
                    p2_back(k - 1)

            # core partial = sum_t mask * term
            tmr = blk.tile([P, ntb], F32, tag="tmr")
            tmc = blk.tile([P, 1], F32, tag="tmc")
            nc.vector.tensor_mul(tmr[:], term[:], mask[:])
            nc.vector.reduce_sum(out=tmc[:], in_=tmr[:], axis=AX.X)
            ps1 = psp.tile([1, 1], F32, tag="ps1")
            nc.tensor.matmul(ps1[:], ones[:], tmc[:])
            nc.vector.tensor_copy(out_sb[0:1, 0:1], ps1[:])
            nc.vector.tensor_copy(out_sb[0:1, 1:3], allr[:])
            nc.vector.tensor_copy(out_sb[0:1, 3:4], alom[0:1, 0:1])
            nc.sync.dma_start(out=out_d[:], in_=out_sb[:])

    nc.compile()
    return nc


def host_prepare(student, teacher, labels, in_dt="e3"):
    """Per-core input maps. Sharding + fp8/bf16 transport cast on host."""
    student = np.asarray(student, dtype=np.float32)
    teacher = np.asarray(teacher, dtype=np.float32)
    labels = np.asarray(labels)
    ntb = TPC // P
    np_dt = {"e3": ml_dtypes.float8_e3m4, "bf16": ml_dtypes.bfloat16,
             "f32": np.float32}[in_dt]
    in_maps = []
    invms = []
    for core in range(NCORES):
        r, h = core // 2, core % 2
        if r % 2 == 0:
            x_full, y_full = teacher[r], student[r]
        else:
            x_full, y_full = student[r], teacher[r]
        sl = slice(h * TPC, (h + 1) * TPC)
        x = np.ascontiguousarray(x_full[sl]).astype(np_dt)
        y = np.ascontiguousarray(y_full[sl]).astype(np_dt)
        t_global = h * TPC + np.arange(TPC)
        valid = t_global <= T - 2
        lbl = np.where(valid, labels[r][np.minimum(t_global + 1, T - 1)], 0)
        m = ((lbl != IGNORE) & valid).astype(np.float32)
        lbl_c = np.clip(lbl, 0, V - 1)
        # gather from the device-visible (cast) values; -1 matches zx = Zx-1
        xlab = x[np.arange(TPC), lbl_c].astype(np.float32) - 1.0
        ylab = y[np.arange(TPC), lbl_c].astype(np.float32) - 1.0
        row_lbl = labels[r][1:]
        mask_total = float(np.maximum((row_lbl != IGNORE).sum(), 1.0))
        invms.append(1.0 / mask_total)

        def fold(vec):
            return np.ascontiguousarray(vec.reshape(ntb, P).T.astype(np.float32))

        in_maps.append({
            "x": x,
            "y": y,
            "xlab": fold(xlab),
            "ylab": fold(ylab),
            "mask": fold(m),
            "invm": np.array([[1.0 / mask_total]], dtype=np.float32),
        })
    return in_maps, invms


def host_combine(results, invms):
    partials = [float(results[i]["out"][0, 0]) for i in range(NCORES)]
    row_vals = []
    for r in range(B):
        pA, pB = partials[2 * r], partials[2 * r + 1]
        row_vals.append(-(pA + pB) * invms[2 * r])
    loss = (2.0 - BETA) * (row_vals[0] + row_vals[2]) / 2.0 \
        + BETA * (row_vals[1] + row_vals[3]) / 2.0
    return np.array(loss, dtype=np.float32)


_NC = None
LAST_RESULT = None  # BassKernelResults from the most recent run (for profiling)


def kernel(student_logits=None, teacher_logits=None, labels=None):
    global _NC, LAST_RESULT
    in_dt = os.environ.get("KERNEL_IN_DT", "e3")
    if _NC is None:
        _NC = build_nc(
            in_dt=in_dt,
            warmup_cc=os.environ.get("KERNEL_WARMUP_CC", "1") == "1",
        )
    in_maps, invms = host_prepare(
        student_logits, teacher_logits, labels, in_dt=in_dt
    )
    res = run_bass_kernel_spmd(
        _NC, in_maps, core_ids=list(range(NCORES)),
        trace=bool(os.environ.get("KERNEL_TRACE")),
    )
    LAST_RESULT = res
    return host_combine(res.results, invms)
